# revision 40
# baseline (speedup 1.0000x reference)
"""Trainium2 Bass kernel for nn_AligningModel (mel/phoneme GLU encoders + soft attention).

Strategy:
  - Data-parallel over batch: 32 samples -> 8 cores x 4 slots.  The sample ->
    (core, slot) assignment starts length-sorted and is then hill-climbed to
    jointly shrink the per-slot compile-time bounds (max mel len AND max
    phoneme len over the 8 cores sharing a slot).
  - Slot-pipelined schedule: slots processed longest-first; attention of slot s
    overlaps the mel GLU stack of slot s+1, so the exposed attention tail is
    the SHORTEST slot.  All phoneme encoders, softmax pad rows, and the big
    pad-region DMA writes happen early (phase 0 / phase 2).
  - fp16 conv path (weights, activations, residual) / bf16 attention path
    (exp tiles need bf16 range: logits can reach +mel_sq since the softmax
    drops the row-constant mel_sq term).  fp16 outputs, upcast on host.
    PSUM accumulation is fp32.
  - All GLU weights are pre-transposed host-side to [i,k,c,o] so each block is
    ONE contiguous 128x6KB DMA, loaded once for the whole kernel; DMA issue
    order puts the startup-critical tensors first.
  - Channel-major [C,T] layout on-chip so the k=3 convs are plain matmuls; the
    sqrt(0.5)^b block scales are folded into the g-path conv weights, and the
    final C^4 into the output copies.
  - Transposes are plain matmuls against a fp16 identity (output lands f32 in
    PSUM, 1 cycle/row).
  - Z (softmax denominator) via ones-columns appended to the time-major ph
    encoding inside the context matmul; exp bias folds -C8*ph_sq and the
    phoneme mask (-1e9).  The padded-row context value is computed from the
    phoneme side alone using sigmoid(bias) == exp(bias) (bias << -19), keeping
    the Exp activation table out of the GLU phase.
  - Engine discipline: PE never waits on an engine that also issues bulk DMA
    descriptors.  gpsimd(Pool) does overflow mask-muls + pad DMA issue, DVE
    does the PSUM-reading elementwise work + residual adds, Act does
    sigmoid/exp + PSUM->SBUF copies, SP(sync) issues all input loads and
    output row writes.
"""

import os
import numpy as np

B = 32
N_CORES = 8
SPC = 4           # samples (slots) per core
T_MEL = 2000
MEL_D = 80
D = 256
C = float(np.sqrt(0.5))
C4 = 0.25         # C**4 exact
C8 = 0.0625       # C**8 exact

_prog_cache = {}


def _chunks(total, cap):
    """Split `total` into <=cap chunks, each a multiple of 4.
    Prefer equal chunks >=256; else greedy cap + remainder."""
    assert total % 4 == 0 and total > 0
    n = -(-total // cap)
    base = min(cap, ((total + n - 1) // n + 3) // 4 * 4)
    if base < 256:
        base = cap
    out = []
    off = 0
    while off < total:
        w = min(base, total - off)
        out.append((off, w))
        off += w
    return out


def _host_prep(mels, phonemes, mel_lens, phoneme_lens, embedding,
               mel_conv_w, mel_conv_b, ph_w, ph_b, mel_w, mel_b, S_pad):
    """Build the per-core input maps (numpy only). Returns (in_maps, flags,
    perm, L, SL) where perm[8*j + c] = original sample index in core c slot j."""
    f32 = np.float32
    f16 = np.float16
    SP2 = S_pad + 2

    ml = np.asarray(mel_lens)
    pl_ = np.asarray(phoneme_lens)
    order = np.argsort(-ml, kind="stable")
    groups = [list(order[8 * j:8 * j + 8]) for j in range(SPC)]

    def slot_cost(g):
        Lm = int(max(ml[i] for i in g))
        Sp = int(max(pl_[i] for i in g))
        Wj = min(T_MEL, -(-(Lm + 2) // 4) * 4)
        SWj = min(S_pad, -(-(Sp + 2) // 4) * 4)
        Tbj = min(T_MEL, -(-(Lm + 2) // 128) * 128)
        nsb = min(S_pad // 128, -(-(Sp + 2) // 128))
        return 102 * Wj + 96 * SWj + (2 + 4.05 * nsb) * Tbj

    # hill-climb swaps between groups to shrink the summed per-slot bounds
    rng = np.random.default_rng(0)
    cur = [slot_cost(g) for g in groups]
    for _ in range(20000):
        a, b = rng.integers(0, SPC, 2)
        if a == b:
            continue
        ia, ib = int(rng.integers(0, 8)), int(rng.integers(0, 8))
        groups[a][ia], groups[b][ib] = groups[b][ib], groups[a][ia]
        na, nb = slot_cost(groups[a]), slot_cost(groups[b])
        if na + nb < cur[a] + cur[b] - 1e-9:
            cur[a], cur[b] = na, nb
        else:
            groups[a][ia], groups[b][ib] = groups[b][ib], groups[a][ia]
    groups.sort(key=lambda g: -max(ml[i] for i in g))
    perm = np.asarray([groups[j][c] for j in range(SPC) for c in range(8)])
    L = tuple(int(max(ml[i] for i in groups[j])) for j in range(SPC))
    SL = tuple(int(max(pl_[i] for i in groups[j])) for j in range(SPC))

    # w0: [k,i,o] -> [i,k,o] contiguous fp16
    w0 = np.ascontiguousarray(
        np.transpose(mel_conv_w, (2, 1, 0)).astype(f32).transpose(1, 0, 2)
    ).astype(f16)

    def pack_w(w4, fold_c4_last=False):
        out = np.empty((4, 128, 3, 2, 512), f16)
        for b in range(4):
            w = np.transpose(w4[b], (2, 1, 0)).astype(f32)  # [k, i, o]
            w = w.reshape(3, 2, 128, 512)
            w[:, :, :, 256:] *= f32(C ** b)
            if fold_c4_last and b == 3:
                # block-3 inputs arrive pre-scaled by C4 (vm4 mask); undo on
                # the g path so sigmoid sees the true gate values.
                w[:, :, :, 256:] *= f32(1.0 / C4)
            out[b] = w.transpose(2, 0, 1, 3).astype(f16)    # [i, k, c, o]
        return np.ascontiguousarray(out)
    wm = pack_w(mel_w, fold_c4_last=True)
    wp = pack_w(ph_w)
    idc = np.eye(128, dtype=f16)

    has_b0 = bool(np.any(mel_conv_b))
    has_bm = bool(np.any(mel_b))
    has_bp = bool(np.any(ph_b))
    shared = {"w0": w0, "wm": wm, "wp": wp, "idc": idc}
    if has_b0:
        shared["b0r"] = mel_conv_b.astype(f32).reshape(1, 256)
    if has_bm:
        bmar_h = mel_b[:, :256].astype(f32).copy()
        bmar_h[3] *= f32(C4)
        shared["bmar"] = bmar_h
        shared["bmg"] = np.ascontiguousarray(
            mel_b[:, 256:].astype(f32).reshape(4, 2, 128).transpose(2, 0, 1).reshape(128, 8))
    if has_bp:
        shared["bpar"] = ph_b[:, :256].astype(f32)
        shared["bpg"] = np.ascontiguousarray(
            ph_b[:, 256:].astype(f32).reshape(4, 2, 128).transpose(2, 0, 1).reshape(128, 8))

    ar = np.arange(T_MEL)
    ars = np.arange(S_pad)
    in_maps = []
    for c in range(N_CORES):
        idx = [int(perm[8 * j + c]) for j in range(SPC)]
        m = dict(shared)
        mcm = np.zeros((SPC, MEL_D, T_MEL + 2), f16)
        vm = np.zeros((SPC, T_MEL + 2), f16)
        vm4 = np.zeros((SPC, T_MEL + 2), f16)
        zph = np.zeros((SPC, 2, 128, SP2), f16)
        vph = np.zeros((SPC, SP2), f16)
        mv = np.full((SPC, S_pad), -1e9, f32)
        for j, b in enumerate(idx):
            mcm[j, :, 1:T_MEL + 1] = np.asarray(mels[b], f32).T.astype(f16)
            vm[j, 1:T_MEL + 1] = (ar < int(mel_lens[b])).astype(f16)
            vm4[j] = vm[j] * f16(C4)
            pl = int(phoneme_lens[b])
            ph_pad = np.concatenate([[0], np.asarray(phonemes[b], np.int64)])[:S_pad]
            e = embedding[ph_pad].astype(f32)
            valid = (ars[:len(e)] <= pl)
            e[~valid] = 0.0
            zph[j, :, :, 1:1 + len(e)] = e.T.reshape(2, 128, len(e)).astype(f16)
            vph[j, 1:1 + len(e)] = valid.astype(f16)
            mv[j, :len(e)][valid] = 0.0
        m["mels_cm"] = mcm
        m["valid_mel"] = vm
        m["valid_mel4"] = vm4
        m["zph0"] = zph
        m["valid_ph"] = vph
        m["mvec"] = mv
        in_maps.append(m)
    return in_maps, (has_b0, has_bm, has_bp), perm, L, SL


def _build_program(S_pad, L, SL, has_b0, has_bm, has_bp):
    from contextlib import ExitStack
    import concourse.bass as bass
    import concourse.bacc as bacc
    import concourse.tile as tile
    from concourse import mybir

    f32 = mybir.dt.float32
    f16 = mybir.dt.float16
    bf16 = mybir.dt.bfloat16
    AF = mybir.ActivationFunctionType
    ALU = mybir.AluOpType
    AX = mybir.AxisListType
    SP2 = S_pad + 2

    # per-slot compile-time bounds (slots sorted longest-first)
    W = [min(T_MEL, -(-(L[j] + 2) // 4) * 4) for j in range(SPC)]       # mel conv cols
    Tb = [min(T_MEL, -(-(L[j] + 2) // 128) * 128) for j in range(SPC)]  # attn rows
    SW = [min(S_pad, -(-(SL[j] + 2) // 4) * 4) for j in range(SPC)]     # ph conv cols
    NSB = [min(S_pad // 128, -(-(SL[j] + 2) // 128)) for j in range(SPC)]
    mel_chunks = [_chunks(W[j], 500 if W[j] > 1000 else
                          (-(--(-W[j] // 4) // 4) * 4 if W[j] > 800 else
                           -(--(-W[j] // 2) // 4) * 4))
                  for j in range(SPC)]
    ph_chunks = [_chunks(SW[j], 512) for j in range(SPC)]
    dot_chunks = [_chunks(Tb[j], 512) for j in range(SPC)]
    YW = [max(W[j], Tb[j]) + 2 for j in range(SPC)]

    nc = bacc.Bacc()
    t_mcm = nc.dram_tensor("mels_cm", [SPC, MEL_D, T_MEL + 2], f16, kind="ExternalInput")
    t_vm = nc.dram_tensor("valid_mel", [SPC, T_MEL + 2], f16, kind="ExternalInput")
    t_vm4 = nc.dram_tensor("valid_mel4", [SPC, T_MEL + 2], f16, kind="ExternalInput")
    t_zph = nc.dram_tensor("zph0", [SPC, 2, 128, SP2], f16, kind="ExternalInput")
    t_vph = nc.dram_tensor("valid_ph", [SPC, SP2], f16, kind="ExternalInput")
    t_mv = nc.dram_tensor("mvec", [SPC, S_pad], f32, kind="ExternalInput")
    t_w0 = nc.dram_tensor("w0", [MEL_D, 3, 256], f16, kind="ExternalInput")
    t_wm = nc.dram_tensor("wm", [4, 128, 3, 2, 512], f16, kind="ExternalInput")
    t_wp = nc.dram_tensor("wp", [4, 128, 3, 2, 512], f16, kind="ExternalInput")
    t_id = nc.dram_tensor("idc", [128, 128], f16, kind="ExternalInput")
    t_b0 = nc.dram_tensor("b0r", [1, 256], f32, kind="ExternalInput") if has_b0 else None
    t_bmar = nc.dram_tensor("bmar", [4, 256], f32, kind="ExternalInput") if has_bm else None
    t_bmg = nc.dram_tensor("bmg", [128, 8], f32, kind="ExternalInput") if has_bm else None
    t_bpar = nc.dram_tensor("bpar", [4, 256], f32, kind="ExternalInput") if has_bp else None
    t_bpg = nc.dram_tensor("bpg", [128, 8], f32, kind="ExternalInput") if has_bp else None
    t_outm = nc.dram_tensor("outm", [SPC, 2, 128, T_MEL], f16, kind="ExternalOutput")
    t_outc = nc.dram_tensor("outc", [SPC, T_MEL, 256], f16, kind="ExternalOutput")

    def bcast(ap, parts):
        return bass.AP(tensor=ap.tensor, offset=ap.offset, ap=[[0, parts]] + list(ap.ap))

    with tile.TileContext(nc) as tc, ExitStack() as ctx:
        wconst = ctx.enter_context(tc.tile_pool(name="wconst", bufs=1))
        ypool = ctx.enter_context(tc.tile_pool(name="y", bufs=2))
        zpool = ctx.enter_context(tc.tile_pool(name="zph", bufs=2))
        vpool = ctx.enter_context(tc.tile_pool(name="vm", bufs=1))
        vppool = ctx.enter_context(tc.tile_pool(name="vph", bufs=1))
        mpool = ctx.enter_context(tc.tile_pool(name="mcm", bufs=1))
        ympool = ctx.enter_context(tc.tile_pool(name="ym", bufs=12))
        ymppool = ctx.enter_context(tc.tile_pool(name="ymp", bufs=6))
        sgpool = ctx.enter_context(tc.tile_pool(name="sig", bufs=8))
        epool = ctx.enter_context(tc.tile_pool(name="exp", bufs=S_pad // 128))
        ztspool = ctx.enter_context(tc.tile_pool(name="zts", bufs=S_pad // 128))
        sqpool = ctx.enter_context(tc.tile_pool(name="sq", bufs=2))
        spool = ctx.enter_context(tc.tile_pool(name="small", bufs=S_pad // 128))
        opool = ctx.enter_context(tc.tile_pool(name="octx", bufs=3))
        padpool = ctx.enter_context(tc.tile_pool(name="padf", bufs=1))
        ppsum = ctx.enter_context(tc.tile_pool(name="pconv", bufs=5, space="PSUM"))
        atpsum = ctx.enter_context(tc.tile_pool(name="pattn", bufs=2, space="PSUM"))
        padpsum = ctx.enter_context(tc.tile_pool(name="ppad", bufs=1, space="PSUM"))

        # ---- constants / weights (weights loaded once; DMA order puts the
        # critical path first: mels(0) -> w0 -> wm/wp block by block) ----
        wm_t = [wconst.tile([128, 3, 2, 512], f16, tag=f"wm{b}", name="wm")
                for b in range(4)]
        wp_t = [wconst.tile([128, 3, 2, 512], f16, tag=f"wp{b}", name="wp")
                for b in range(4)]
        id_t = wconst.tile([128, 128], f16, tag="id")
        w0_t = wconst.tile([MEL_D, 3, 256], f16, tag="w0")
        zero_t = wconst.tile([128, 1536], f16, tag="zero")
        nc.vector.memset(zero_t[:], 0.0)
        ones1_t = wconst.tile([1, 128], bf16, tag="ones1")
        nc.vector.memset(ones1_t[:], 1.0)

        def load_weights_head():
            # separate hw queue from the mel loads so both ramp concurrently
            nc.scalar.dma_start(out=w0_t[:], in_=t_w0[:])
            nc.scalar.dma_start(out=wm_t[0][:], in_=t_wm[0])

        def load_weights_rest():
            for b in range(1, 4):
                nc.sync.dma_start(out=wm_t[b][:], in_=t_wm[b])
                nc.sync.dma_start(out=wp_t[b][:], in_=t_wp[b])

        need_ones = has_b0 or has_bm or has_bp
        if need_ones:
            ones_t = wconst.tile([1, 512], f32, tag="ones")
            nc.vector.memset(ones_t[:], 1.0)
        if has_b0:
            b0_t = wconst.tile([1, 256], f32, tag="b0")
            nc.sync.dma_start(out=b0_t[:], in_=t_b0[:])
        if has_bm:
            bmar_t = wconst.tile([4, 256], f32, tag="bmar")
            nc.sync.dma_start(out=bmar_t[:], in_=t_bmar[:])
            bmg_t = wconst.tile([128, 8], f32, tag="bmg")
            nc.sync.dma_start(out=bmg_t[:], in_=t_bmg[:])
        if has_bp:
            bpar_t = wconst.tile([4, 256], f32, tag="bpar")
            nc.sync.dma_start(out=bpar_t[:], in_=t_bpar[:])
            bpg_t = wconst.tile([128, 8], f32, tag="bpg")
            nc.sync.dma_start(out=bpg_t[:], in_=t_bpg[:])

        state = {}   # s -> dict of tiles

        def load_mel(s, chunked=False):
            mc = mpool.tile([MEL_D, W[s] + 2], f16, tag=f"mc{s}", name="mcm")
            if chunked:
                lo = 0
                for (off, n) in mel_chunks[s]:
                    hi = off + n + 2
                    nc.sync.dma_start(out=mc[:, lo:hi], in_=t_mcm[s, :, lo:hi])
                    lo = hi
            else:
                nc.sync.dma_start(out=mc[:], in_=t_mcm[s, :, 0:W[s] + 2])
            vb = vpool.tile([128, W[s] + 2], f16, tag=f"vm{s}", name="vm")
            nc.sync.dma_start(out=vb[:], in_=bcast(t_vm[s, 0:W[s] + 2], 128))
            vb4 = vpool.tile([128, W[s] + 2], f16, tag=f"vm4{s}", name="vm4")
            nc.sync.dma_start(out=vb4[:], in_=bcast(t_vm4[s, 0:W[s] + 2], 128))
            state.setdefault(s, {})
            state[s]["mc"] = mc
            state[s]["vb"] = vb
            state[s]["vb4"] = vb4

        def load_ph(s, eng=None):
            eng = eng or nc.sync
            zt = [zpool.tile([128, SP2], f16, tag=f"z{s}", name="zph")
                  for _ in range(2)]
            for icb in range(2):
                eng.dma_start(out=zt[icb][:], in_=t_zph[s, icb])
            vpb = vppool.tile([128, SP2], f16, tag=f"vp{s}", name="vph")
            eng.dma_start(out=vpb[:], in_=bcast(t_vph[s], 128))
            state.setdefault(s, {})
            state[s]["zt"] = zt
            state[s]["vpb"] = vpb

        def init_conv(s):
            st = state[s]
            mc = st["mc"]
            yt = [ypool.tile([128, YW[s]], f16, tag=f"y{s}", name="y")
                  for _ in range(2)]
            for icb in range(2):
                nc.vector.memset(yt[icb][:, 0:1], 0.0)
                if 1 + W[s] < YW[s]:
                    nc.vector.memset(yt[icb][:, 1 + W[s]:YW[s]], 0.0)
            for (off, n) in mel_chunks[s]:
                for ocb in range(2):
                    pi = ppsum.tile([128, 512], f32, tag="cps", name="cps")
                    for k in range(3):
                        nc.tensor.matmul(
                            pi[:, :n],
                            w0_t[:, k, 128 * ocb:128 * ocb + 128],
                            mc[:, off + k:off + k + n],
                            start=(k == 0), stop=(k == 2 and not has_b0))
                    if has_b0:
                        nc.tensor.matmul(pi[:, :n],
                                         b0_t[0:1, 128 * ocb:128 * ocb + 128],
                                         ones_t[0:1, :n],
                                         start=False, stop=True)
                    nc.vector.tensor_copy(out=yt[ocb][:, off + 1:off + 1 + n],
                                          in_=pi[:, :n])
            st["yt"] = yt

        def glu_block(y_tiles, ym_tag, ym_pool, width, chunks, wt,
                      bar_t, bg_t, blk, vb):
            """One GLU block, channel-major, fp16, in-place on y_tiles."""
            yms = {}
            for icb in range(2):
                for ci, (off, n) in enumerate(chunks):
                    ym = ym_pool.tile([128, width], f16, tag=ym_tag, name=ym_tag)
                    eng = nc.vector if ci < 3 else nc.gpsimd
                    eng.tensor_mul(out=ym[:, :n + 2],
                                   in0=y_tiles[icb][:, off:off + n + 2],
                                   in1=vb[:, off:off + n + 2])
                    yms[(icb, off)] = ym
            for cpair in range(0, len(chunks), 2):
                sub = chunks[cpair:cpair + 2]
                for oco in range(2):
                    pa = {}
                    pg = {}
                    for (off, n) in sub:
                        pa[off] = ppsum.tile([128, 512], f32, tag="cps", name="cps")
                        pg[off] = ppsum.tile([128, 512], f32, tag="cps", name="cps")
                    last_mm = (2, 1)
                    for k in range(3):
                        for icb in range(2):
                            st_ = (k == 0 and icb == 0)
                            sp = ((k, icb) == last_mm and bar_t is None)
                            wa = wt[:, k, icb, 128 * oco:128 * oco + 128]
                            wg = wt[:, k, icb, 256 + 128 * oco:384 + 128 * oco]
                            for (off, n) in sub:
                                nc.tensor.matmul(pa[off][:, :n], wa,
                                                 yms[(icb, off)][:, k:k + n],
                                                 start=st_, stop=sp)
                            for (off, n) in sub:
                                nc.tensor.matmul(pg[off][:, :n], wg,
                                                 yms[(icb, off)][:, k:k + n],
                                                 start=st_,
                                                 stop=((k, icb) == last_mm))
                    if bar_t is not None:
                        for (off, n) in sub:
                            nc.tensor.matmul(pa[off][:, :n],
                                             bar_t[blk:blk + 1, 128 * oco:128 * oco + 128],
                                             ones_t[0:1, :n],
                                             start=False, stop=True)
                    for (off, n) in sub:
                        sig = sgpool.tile([128, 512], f16, tag="sig", name="sig")
                        bias = bg_t[:, 2 * blk + oco:2 * blk + oco + 1] if bg_t is not None else 0.0
                        nc.scalar.activation(out=sig[:, :n], in_=pg[off][:, :n],
                                             func=AF.Sigmoid, bias=bias)
                        nc.vector.tensor_mul(out=sig[:, :n], in0=pa[off][:, :n],
                                             in1=sig[:, :n])
                        nc.vector.tensor_add(out=y_tiles[oco][:, off + 1:off + 1 + n],
                                             in0=sig[:, :n],
                                             in1=yms[(oco, off)][:, 1:1 + n])

        def glu_mel(s, blk):
            st = state[s]
            glu_block(st["yt"], "ym", ympool, 502, mel_chunks[s], wm_t[blk],
                      bmar_t if has_bm else None,
                      bmg_t if has_bm else None, blk,
                      st["vb4"] if blk == 3 else st["vb"])

        def glu_ph(s, blk):
            st = state[s]
            glu_block(st["zt"], "ymp", ymppool, SP2, ph_chunks[s], wp_t[blk],
                      bpar_t if has_bp else None,
                      bpg_t if has_bp else None, blk, st["vpb"])

        def attn_pre(s):
            """After ph GLU of slot s: transpose z, biases, ctx pad row + pad DMA."""
            st = state[s]
            zt = st["zt"]
            n_sb = NSB[s]
            mv_t = spool.tile([128, S_pad // 128], f32, tag=f"mv{s}", name="mv")
            src = t_mv[s]
            nc.sync.dma_start(out=mv_t[:, :n_sb], in_=bass.AP(
                tensor=src.tensor, offset=src.offset,
                ap=[[1, 128], [128, n_sb]]))
            zts = []
            biases = []
            for sb in range(n_sb):
                zp = atpsum.tile([128, 512], f32, tag="atp", name="ztp")
                for dcb in range(2):
                    nc.tensor.matmul(zp[:, 128 * dcb:128 * dcb + 128],
                                     zt[dcb][:, 1 + 128 * sb:129 + 128 * sb],
                                     id_t[:], start=True, stop=True)
                z = ztspool.tile([128, 260], bf16, tag=f"zts{s}", name="zts")
                nc.vector.tensor_copy(out=z[:, 0:256], in_=zp[:, 0:256])
                nc.vector.memset(z[:, 256:260], 1.0)
                sq = sqpool.tile([128, 256], f32, tag="sq", name="sq")
                nc.gpsimd.tensor_mul(out=sq[:], in0=z[:, 0:256], in1=z[:, 0:256])
                ph2 = spool.tile([128, 1], f32, tag="phsq", name="phsq")
                nc.vector.tensor_reduce(out=ph2[:], in_=sq[:], axis=AX.X, op=ALU.add)
                bias_sb = spool.tile([128, 1], f32, tag=f"bias{s}", name="bias")
                nc.vector.tensor_scalar(out=bias_sb[:], in0=ph2[:],
                                        scalar1=-C8, scalar2=mv_t[:, sb:sb + 1],
                                        op0=ALU.mult, op1=ALU.add)
                zts.append(z)
                biases.append(bias_sb)
            st["zts"] = zts
            st["biases"] = biases
            if Tb[s] < T_MEL:
                # ctx pad row (softmax with zero mel row) from the ph side
                # only.  sigmoid(x) == exp(x) to ~e^-19 relative for x <= -19;
                # biases are -C8*|ph|^2 (or -1e9), always << -19, and Sigmoid
                # is the resident table during the GLU stack.
                pp = padpsum.tile([128, 260], f32, tag="padp", name="padp")
                for sb in range(n_sb):
                    eb = spool.tile([128, 1], bf16, tag="eb", name="eb")
                    nc.scalar.activation(out=eb[:], in_=biases[sb][:],
                                         func=AF.Sigmoid)
                    nc.tensor.matmul(pp[0:1, :260], eb[:], zts[sb][:],
                                     start=(sb == 0), stop=(sb == n_sb - 1))
                rc = spool.tile([1, 1], f32, tag="prc", name="prc")
                nc.vector.reciprocal(out=rc[:], in_=pp[0:1, 256:257])
                prow = spool.tile([1, 256], bf16, tag=f"prow{s}", name="prow")
                nc.vector.tensor_scalar(out=prow[:], in0=pp[0:1, 0:256],
                                        scalar1=rc[:], scalar2=C4,
                                        op0=ALU.mult, op1=ALU.mult)
                pr = padpsum.tile([128, 260], f32, tag="padp", name="padp")
                nc.tensor.matmul(pr[:, :256], ones1_t[:], prow[:],
                                 start=True, stop=True)
                padf = padpool.tile([128, 256], f16, tag=f"padf{s}", name="padf")
                nc.vector.tensor_copy(out=padf[:], in_=pr[:, :256])
                for r0 in range(Tb[s], T_MEL, 128):
                    nr = min(128, T_MEL - r0)
                    nc.gpsimd.dma_start(out=t_outc[s, r0:r0 + nr, :],
                                        in_=padf[:nr, :])

        def zero_pad_mel(s):
            if Tb[s] >= T_MEL:
                return
            nz = T_MEL - Tb[s]
            for dcb in range(2):
                nc.gpsimd.dma_start(out=t_outm[s, dcb, :, Tb[s]:T_MEL],
                                    in_=zero_t[:, :nz])

        def dots(s):
            st = state[s]
            yt, zt, biases = st["yt"], st["zt"], st["biases"]
            n_sb = NSB[s]
            ets = []
            for sb in range(n_sb):
                et = epool.tile([128, Tb[s]], bf16, tag=f"et{s}", name="exp")
                for (off, n) in dot_chunks[s]:
                    dp = atpsum.tile([128, 512], f32, tag="atp", name="dps")
                    for dcb in range(2):
                        nc.tensor.matmul(
                            dp[:, :n],
                            zt[dcb][:, 1 + 128 * sb:129 + 128 * sb],
                            yt[dcb][:, 1 + off:1 + off + n],
                            start=(dcb == 0), stop=(dcb == 1))
                    nc.scalar.activation(out=et[:, off:off + n], in_=dp[:, :n],
                                         func=AF.Exp, bias=biases[sb], scale=2.0)
                ets.append(et)
            st["ets"] = ets

        def ctx(s):
            st = state[s]
            ets, zts = st["ets"], st["zts"]
            n_sb = NSB[s]
            for tt in range((Tb[s] + 127) // 128):
                rows = min(128, Tb[s] - 128 * tt)
                cp = atpsum.tile([128, 512], f32, tag="atp", name="cxs")
                for sb in range(n_sb):
                    nc.tensor.matmul(cp[:rows, :260],
                                     ets[sb][:, 128 * tt:128 * tt + rows],
                                     zts[sb][:],
                                     start=(sb == 0), stop=(sb == n_sb - 1))
                rc = spool.tile([128, 1], f32, tag="rc", name="rc")
                nc.vector.reciprocal(out=rc[:rows], in_=cp[:rows, 256:257])
                oc = opool.tile([128, 256], f16, tag="oc", name="oc")
                nc.vector.tensor_scalar(out=oc[:rows, :],
                                        in0=cp[:rows, 0:256],
                                        scalar1=rc[:rows], scalar2=C4,
                                        op0=ALU.mult, op1=ALU.mult)
                eng = nc.scalar if s == SPC - 1 else nc.sync
                eng.dma_start(out=t_outc[s, 128 * tt:128 * tt + rows, :],
                              in_=oc[:rows, :])

        def mel_out(s):
            st = state[s]
            yt = st["yt"]
            eng = nc.scalar if s == SPC - 1 else nc.sync
            for dcb in range(2):
                eng.dma_start(out=t_outm[s, dcb, :, 0:Tb[s]],
                              in_=yt[dcb][:, 1:1 + Tb[s]])

        # ================= schedule =================
        # phase 0: slot-0 mel GLU x ph(0), then a 4-stream round-robin of
        # ph(1..3) + mel(1) so the PE always has an independent stream while
        # each GLU block's sigmoid/mul/add tail drains.  attn_pre(s) runs
        # right after ph(s) finishes so its DVE/Act work spreads out.
        load_mel(0, chunked=True)
        load_weights_head()
        load_ph(0)
        nc.scalar.dma_start(out=wp_t[0][:], in_=t_wp[0])
        nc.scalar.dma_start(out=id_t[:], in_=t_id[:])
        load_mel(1)
        load_weights_rest()
        for s in range(1, SPC):
            load_ph(s)
        init_conv(0)
        for blk in range(4):
            glu_mel(0, blk)
            glu_ph(0, blk)
        attn_pre(0)
        for blk in range(4):
            for s in range(1, SPC):
                glu_ph(s, blk)
                if blk == 3:
                    attn_pre(s)

        # phases 1..3: attn(s-1) overlapped with mel GLU(s)
        for s in range(1, SPC):
            if s + 1 < SPC:
                load_mel(s + 1)
            dots(s - 1)
            init_conv(s)
            mel_out(s - 1)
            glu_mel(s, 0)
            ctx(s - 1)
            if s == 2:
                for s2 in range(SPC):
                    zero_pad_mel(s2)
            for blk in range(1, 4):
                glu_mel(s, blk)
        # tail: attention of the shortest slot
        dots(SPC - 1)
        mel_out(SPC - 1)
        ctx(SPC - 1)

    if not nc.is_finalized():
        nc.finalize()
    return nc


def _get_program(S_pad, L, SL, has_b0, has_bm, has_bp):
    key = (S_pad, L, SL, has_b0, has_bm, has_bp)
    if key not in _prog_cache:
        _prog_cache[key] = _build_program(S_pad, L, SL, has_b0, has_bm, has_bp)
    return _prog_cache[key]


LAST_RESULTS = None


def _install_ntff_hook():
    """Provide antenv.axon_hooks (missing in this image) so trace=True works."""
    import sys
    import types
    import ctypes
    import contextlib
    if "antenv.axon_hooks" in sys.modules:
        return
    try:
        import antenv
    except ImportError:
        return
    mod = types.ModuleType("antenv.axon_hooks")
    state = {}
    mod.set_axon_ntff_profile_hook = lambda h: state.__setitem__("h", h)
    mod.get_axon_ntff_profile_hook = lambda: state.get("h")
    sys.modules["antenv.axon_hooks"] = mod
    antenv.axon_hooks = mod
    so_path = "/opt/axon/libaxon_pjrt.so"
    if not os.path.exists(so_path):
        return
    lib = ctypes.CDLL(so_path)
    if not hasattr(lib, "axon_start_nrt_profile"):
        return
    lib.axon_start_nrt_profile.argtypes = [ctypes.POINTER(ctypes.c_int64),
                                           ctypes.c_size_t]
    lib.axon_start_nrt_profile.restype = ctypes.c_int64
    lib.axon_stop_nrt_profile.argtypes = [ctypes.c_char_p]
    lib.axon_stop_nrt_profile.restype = ctypes.c_int64

    @contextlib.contextmanager
    def _hook(output_dir, device_ids):
        import jax
        jax.devices()
        if device_ids:
            ids = (ctypes.c_int64 * len(device_ids))(*device_ids)
            rc = lib.axon_start_nrt_profile(ids, len(device_ids))
        else:
            rc = lib.axon_start_nrt_profile(None, 0)
        if rc != 0:
            raise RuntimeError(f"axon_start_nrt_profile rc={rc}")
        try:
            yield
        finally:
            n = lib.axon_stop_nrt_profile(str(output_dir).encode())
            print(f"ntff profile: {n} file(s) -> {output_dir}")

    mod.set_axon_ntff_profile_hook(_hook)


def kernel(mels, phonemes, mel_lens, phoneme_lens, embedding,
           mel_conv_w, mel_conv_b, ph_w, ph_b, mel_w, mel_b):
    global LAST_RESULTS
    from concourse.bass_utils import run_bass_kernel_spmd


    mels = np.asarray(mels)
    assert mels.shape == (B, T_MEL, MEL_D), mels.shape
    max_pl = int(np.max(np.asarray(phoneme_lens)))
    S_pad = 512 if max_pl <= 511 else 640

    in_maps, flags, perm, L, SL = _host_prep(
        np.asarray(mels), np.asarray(phonemes), np.asarray(mel_lens),
        np.asarray(phoneme_lens), np.asarray(embedding),
        np.asarray(mel_conv_w), np.asarray(mel_conv_b),
        np.asarray(ph_w), np.asarray(ph_b),
        np.asarray(mel_w), np.asarray(mel_b), S_pad)

    nc = _get_program(S_pad, L, SL, *flags)
    trace = bool(int(os.environ.get("KERNEL_TRACE", "0")))
    if trace:
        _install_ntff_hook()
    res = run_bass_kernel_spmd(nc, in_maps, core_ids=list(range(N_CORES)),
                               trace=trace,
                               tmpdir=os.environ.get("KERNEL_TRACE_DIR"))
    LAST_RESULTS = res
    out = np.empty((B, T_MEL, 512), np.float32)
    for c in range(N_CORES):
        for j in range(SPC):
            b = int(perm[8 * j + c])
            out[b, :, :256] = res.results[c]["outm"][j].reshape(256, T_MEL).T
            out[b, :, 256:] = res.results[c]["outc"][j]
    return out


# revision 41
# speedup vs baseline: 1.0162x; 1.0162x over previous
"""Trainium2 Bass kernel for nn_AligningModel (mel/phoneme GLU encoders + soft attention).

Strategy:
  - Data-parallel over batch: 32 samples -> 8 cores x 4 slots.  The sample ->
    (core, slot) assignment starts length-sorted and is then hill-climbed to
    jointly shrink the per-slot compile-time bounds (max mel len AND max
    phoneme len over the 8 cores sharing a slot).
  - Slot-pipelined schedule: slots processed longest-first; attention of slot s
    overlaps the mel GLU stack of slot s+1, so the exposed attention tail is
    the SHORTEST slot.  All phoneme encoders, softmax pad rows, and the big
    pad-region DMA writes happen early (phase 0 / phase 2).
  - fp16 conv path (weights, activations, residual) / bf16 attention path
    (exp tiles need bf16 range: logits can reach +mel_sq since the softmax
    drops the row-constant mel_sq term).  fp16 outputs, upcast on host.
    PSUM accumulation is fp32.
  - All GLU weights are pre-transposed host-side to [i,k,c,o] so each block is
    ONE contiguous 128x6KB DMA, loaded once for the whole kernel; DMA issue
    order puts the startup-critical tensors first.
  - Channel-major [C,T] layout on-chip so the k=3 convs are plain matmuls; the
    sqrt(0.5)^b block scales are folded into the g-path conv weights, and the
    final C^4 into the output copies.
  - Transposes are plain matmuls against a fp16 identity (output lands f32 in
    PSUM, 1 cycle/row).
  - Z (softmax denominator) via ones-columns appended to the time-major ph
    encoding inside the context matmul; exp bias folds -C8*ph_sq and the
    phoneme mask (-1e9).  The padded-row context value is computed from the
    phoneme side alone using sigmoid(bias) == exp(bias) (bias << -19), keeping
    the Exp activation table out of the GLU phase.
  - Engine discipline: PE never waits on an engine that also issues bulk DMA
    descriptors.  gpsimd(Pool) does overflow mask-muls + pad DMA issue, DVE
    does the PSUM-reading elementwise work + residual adds, Act does
    sigmoid/exp + PSUM->SBUF copies, SP(sync) issues all input loads and
    output row writes.
"""

import os
import numpy as np

B = 32
N_CORES = 8
SPC = 4           # samples (slots) per core
T_MEL = 2000
MEL_D = 80
D = 256
C = float(np.sqrt(0.5))
C4 = 0.25         # C**4 exact
C8 = 0.0625       # C**8 exact

_prog_cache = {}


def _chunks(total, cap):
    """Split `total` into <=cap chunks, each a multiple of 4.
    Prefer equal chunks >=256; else greedy cap + remainder."""
    assert total % 4 == 0 and total > 0
    n = -(-total // cap)
    base = min(cap, ((total + n - 1) // n + 3) // 4 * 4)
    if base < 256:
        base = cap
    out = []
    off = 0
    while off < total:
        w = min(base, total - off)
        out.append((off, w))
        off += w
    return out


def _host_prep(mels, phonemes, mel_lens, phoneme_lens, embedding,
               mel_conv_w, mel_conv_b, ph_w, ph_b, mel_w, mel_b, S_pad):
    """Build the per-core input maps (numpy only). Returns (in_maps, flags,
    perm, L, SL) where perm[8*j + c] = original sample index in core c slot j."""
    f32 = np.float32
    f16 = np.float16
    SP2 = S_pad + 2

    ml = np.asarray(mel_lens)
    pl_ = np.asarray(phoneme_lens)
    order = np.argsort(-ml, kind="stable")
    groups = [list(order[8 * j:8 * j + 8]) for j in range(SPC)]

    def slot_cost(g):
        Lm = int(max(ml[i] for i in g))
        Sp = int(max(pl_[i] for i in g))
        Wj = min(T_MEL, -(-(Lm + 2) // 4) * 4)
        SWj = min(S_pad, -(-(Sp + 2) // 4) * 4)
        Tbj = min(T_MEL, -(-(Lm + 2) // 128) * 128)
        nsb = min(S_pad // 128, -(-(Sp + 2) // 128))
        return 102 * Wj + 96 * SWj + (2 + 4.05 * nsb) * Tbj

    # hill-climb swaps between groups to shrink the summed per-slot bounds
    rng = np.random.default_rng(0)
    cur = [slot_cost(g) for g in groups]
    for _ in range(20000):
        a, b = rng.integers(0, SPC, 2)
        if a == b:
            continue
        ia, ib = int(rng.integers(0, 8)), int(rng.integers(0, 8))
        groups[a][ia], groups[b][ib] = groups[b][ib], groups[a][ia]
        na, nb = slot_cost(groups[a]), slot_cost(groups[b])
        if na + nb < cur[a] + cur[b] - 1e-9:
            cur[a], cur[b] = na, nb
        else:
            groups[a][ia], groups[b][ib] = groups[b][ib], groups[a][ia]
    groups.sort(key=lambda g: -max(ml[i] for i in g))
    perm = np.asarray([groups[j][c] for j in range(SPC) for c in range(8)])
    L = tuple(int(max(ml[i] for i in groups[j])) for j in range(SPC))
    SL = tuple(int(max(pl_[i] for i in groups[j])) for j in range(SPC))

    # w0: [k,i,o] -> [i,k,o] contiguous fp16
    w0 = np.ascontiguousarray(
        np.transpose(mel_conv_w, (2, 1, 0)).astype(f32).transpose(1, 0, 2)
    ).astype(f16)

    def pack_w(w4, fold_c4_last=False):
        out = np.empty((4, 128, 3, 2, 512), f16)
        for b in range(4):
            w = np.transpose(w4[b], (2, 1, 0)).astype(f32)  # [k, i, o]
            w = w.reshape(3, 2, 128, 512)
            w[:, :, :, 256:] *= f32(C ** b)
            if fold_c4_last and b == 3:
                # block-3 inputs arrive pre-scaled by C4 (vm4 mask); undo on
                # the g path so sigmoid sees the true gate values.
                w[:, :, :, 256:] *= f32(1.0 / C4)
            out[b] = w.transpose(2, 0, 1, 3).astype(f16)    # [i, k, c, o]
        return np.ascontiguousarray(out)
    wm = pack_w(mel_w, fold_c4_last=True)
    wp = pack_w(ph_w)
    idc = np.eye(128, dtype=f16)

    has_b0 = bool(np.any(mel_conv_b))
    has_bm = bool(np.any(mel_b))
    has_bp = bool(np.any(ph_b))
    shared = {"w0": w0, "wm": wm, "wp": wp, "idc": idc}
    if has_b0:
        shared["b0r"] = mel_conv_b.astype(f32).reshape(1, 256)
    if has_bm:
        bmar_h = mel_b[:, :256].astype(f32).copy()
        bmar_h[3] *= f32(C4)
        shared["bmar"] = bmar_h
        shared["bmg"] = np.ascontiguousarray(
            mel_b[:, 256:].astype(f32).reshape(4, 2, 128).transpose(2, 0, 1).reshape(128, 8))
    if has_bp:
        shared["bpar"] = ph_b[:, :256].astype(f32)
        shared["bpg"] = np.ascontiguousarray(
            ph_b[:, 256:].astype(f32).reshape(4, 2, 128).transpose(2, 0, 1).reshape(128, 8))

    ar = np.arange(T_MEL)
    ars = np.arange(S_pad)
    in_maps = []
    for c in range(N_CORES):
        idx = [int(perm[8 * j + c]) for j in range(SPC)]
        m = dict(shared)
        mcm = np.zeros((SPC, MEL_D, T_MEL + 2), f16)
        vm = np.zeros((SPC, T_MEL + 2), f16)
        vm4 = np.zeros((SPC, T_MEL + 2), f16)
        zph = np.zeros((SPC, 2, 128, SP2), f16)
        vph = np.zeros((SPC, SP2), f16)
        mv = np.full((SPC, S_pad), -1e9, f32)
        for j, b in enumerate(idx):
            mcm[j, :, 1:T_MEL + 1] = np.asarray(mels[b], f32).T.astype(f16)
            vm[j, 1:T_MEL + 1] = (ar < int(mel_lens[b])).astype(f16)
            vm4[j] = vm[j] * f16(C4)
            pl = int(phoneme_lens[b])
            ph_pad = np.concatenate([[0], np.asarray(phonemes[b], np.int64)])[:S_pad]
            e = embedding[ph_pad].astype(f32)
            valid = (ars[:len(e)] <= pl)
            e[~valid] = 0.0
            zph[j, :, :, 1:1 + len(e)] = e.T.reshape(2, 128, len(e)).astype(f16)
            vph[j, 1:1 + len(e)] = valid.astype(f16)
            mv[j, :len(e)][valid] = 0.0
        m["mels_cm"] = mcm
        m["valid_mel"] = vm
        m["valid_mel4"] = vm4
        m["zph0"] = zph
        m["valid_ph"] = vph
        m["mvec"] = mv
        in_maps.append(m)
    return in_maps, (has_b0, has_bm, has_bp), perm, L, SL


def _build_program(S_pad, L, SL, has_b0, has_bm, has_bp):
    from contextlib import ExitStack
    import concourse.bass as bass
    import concourse.bacc as bacc
    import concourse.tile as tile
    from concourse import mybir

    f32 = mybir.dt.float32
    f16 = mybir.dt.float16
    bf16 = mybir.dt.bfloat16
    AF = mybir.ActivationFunctionType
    ALU = mybir.AluOpType
    AX = mybir.AxisListType
    SP2 = S_pad + 2

    # per-slot compile-time bounds (slots sorted longest-first)
    W = [min(T_MEL, -(-(L[j] + 2) // 4) * 4) for j in range(SPC)]       # mel conv cols
    Tb = [min(T_MEL, -(-(L[j] + 2) // 128) * 128) for j in range(SPC)]  # attn rows
    SW = [min(S_pad, -(-(SL[j] + 2) // 4) * 4) for j in range(SPC)]     # ph conv cols
    NSB = [min(S_pad // 128, -(-(SL[j] + 2) // 128)) for j in range(SPC)]
    mel_chunks = [_chunks(W[j], 500 if W[j] > 1000 else
                          (-(--(-W[j] // 4) // 4) * 4 if W[j] > 800 else
                           -(--(-W[j] // 2) // 4) * 4))
                  for j in range(SPC)]
    ph_chunks = [_chunks(SW[j], 512) for j in range(SPC)]
    dot_chunks = [_chunks(Tb[j], 512) for j in range(SPC)]
    YW = [max(W[j], Tb[j]) + 2 for j in range(SPC)]

    nc = bacc.Bacc()
    t_mcm = nc.dram_tensor("mels_cm", [SPC, MEL_D, T_MEL + 2], f16, kind="ExternalInput")
    t_vm = nc.dram_tensor("valid_mel", [SPC, T_MEL + 2], f16, kind="ExternalInput")
    t_vm4 = nc.dram_tensor("valid_mel4", [SPC, T_MEL + 2], f16, kind="ExternalInput")
    t_zph = nc.dram_tensor("zph0", [SPC, 2, 128, SP2], f16, kind="ExternalInput")
    t_vph = nc.dram_tensor("valid_ph", [SPC, SP2], f16, kind="ExternalInput")
    t_mv = nc.dram_tensor("mvec", [SPC, S_pad], f32, kind="ExternalInput")
    t_w0 = nc.dram_tensor("w0", [MEL_D, 3, 256], f16, kind="ExternalInput")
    t_wm = nc.dram_tensor("wm", [4, 128, 3, 2, 512], f16, kind="ExternalInput")
    t_wp = nc.dram_tensor("wp", [4, 128, 3, 2, 512], f16, kind="ExternalInput")
    t_id = nc.dram_tensor("idc", [128, 128], f16, kind="ExternalInput")
    t_b0 = nc.dram_tensor("b0r", [1, 256], f32, kind="ExternalInput") if has_b0 else None
    t_bmar = nc.dram_tensor("bmar", [4, 256], f32, kind="ExternalInput") if has_bm else None
    t_bmg = nc.dram_tensor("bmg", [128, 8], f32, kind="ExternalInput") if has_bm else None
    t_bpar = nc.dram_tensor("bpar", [4, 256], f32, kind="ExternalInput") if has_bp else None
    t_bpg = nc.dram_tensor("bpg", [128, 8], f32, kind="ExternalInput") if has_bp else None
    t_outm = nc.dram_tensor("outm", [SPC, 2, 128, T_MEL], f16, kind="ExternalOutput")
    t_outc = nc.dram_tensor("outc", [SPC, T_MEL, 256], f16, kind="ExternalOutput")

    def bcast(ap, parts):
        return bass.AP(tensor=ap.tensor, offset=ap.offset, ap=[[0, parts]] + list(ap.ap))

    with tile.TileContext(nc) as tc, ExitStack() as ctx:
        wconst = ctx.enter_context(tc.tile_pool(name="wconst", bufs=1))
        ypool = ctx.enter_context(tc.tile_pool(name="y", bufs=2))
        zpool = ctx.enter_context(tc.tile_pool(name="zph", bufs=2))
        vpool = ctx.enter_context(tc.tile_pool(name="vm", bufs=1))
        vppool = ctx.enter_context(tc.tile_pool(name="vph", bufs=1))
        mpool = ctx.enter_context(tc.tile_pool(name="mcm", bufs=1))
        ympool = ctx.enter_context(tc.tile_pool(name="ym", bufs=12))
        ymppool = ctx.enter_context(tc.tile_pool(name="ymp", bufs=6))
        sgpool = ctx.enter_context(tc.tile_pool(name="sig", bufs=8))
        epool = ctx.enter_context(tc.tile_pool(name="exp", bufs=S_pad // 128))
        ztspool = ctx.enter_context(tc.tile_pool(name="zts", bufs=S_pad // 128))
        sqpool = ctx.enter_context(tc.tile_pool(name="sq", bufs=2))
        spool = ctx.enter_context(tc.tile_pool(name="small", bufs=S_pad // 128))
        opool = ctx.enter_context(tc.tile_pool(name="octx", bufs=3))
        padpool = ctx.enter_context(tc.tile_pool(name="padf", bufs=1))
        ppsum = ctx.enter_context(tc.tile_pool(name="pconv", bufs=5, space="PSUM"))
        atpsum = ctx.enter_context(tc.tile_pool(name="pattn", bufs=2, space="PSUM"))
        padpsum = ctx.enter_context(tc.tile_pool(name="ppad", bufs=1, space="PSUM"))

        # ---- constants / weights (weights loaded once; DMA order puts the
        # critical path first: mels(0) -> w0 -> wm/wp block by block) ----
        wm_t = [wconst.tile([128, 3, 2, 512], f16, tag=f"wm{b}", name="wm")
                for b in range(4)]
        wp_t = [wconst.tile([128, 3, 2, 512], f16, tag=f"wp{b}", name="wp")
                for b in range(4)]
        id_t = wconst.tile([128, 128], f16, tag="id")
        w0_t = wconst.tile([MEL_D, 3, 256], f16, tag="w0")
        zero_t = wconst.tile([128, 1536], f16, tag="zero")
        nc.vector.memset(zero_t[:], 0.0)
        ones1_t = wconst.tile([1, 128], bf16, tag="ones1")
        nc.vector.memset(ones1_t[:], 1.0)

        def load_weights_head():
            # separate hw queue from the mel loads so both ramp concurrently
            nc.scalar.dma_start(out=w0_t[:], in_=t_w0[:])
            nc.scalar.dma_start(out=wm_t[0][:], in_=t_wm[0])

        def load_weights_rest():
            for b in range(1, 4):
                nc.sync.dma_start(out=wm_t[b][:], in_=t_wm[b])
                nc.sync.dma_start(out=wp_t[b][:], in_=t_wp[b])

        need_ones = has_b0 or has_bm or has_bp
        if need_ones:
            ones_t = wconst.tile([1, 512], f32, tag="ones")
            nc.vector.memset(ones_t[:], 1.0)
        if has_b0:
            b0_t = wconst.tile([1, 256], f32, tag="b0")
            nc.sync.dma_start(out=b0_t[:], in_=t_b0[:])
        if has_bm:
            bmar_t = wconst.tile([4, 256], f32, tag="bmar")
            nc.sync.dma_start(out=bmar_t[:], in_=t_bmar[:])
            bmg_t = wconst.tile([128, 8], f32, tag="bmg")
            nc.sync.dma_start(out=bmg_t[:], in_=t_bmg[:])
        if has_bp:
            bpar_t = wconst.tile([4, 256], f32, tag="bpar")
            nc.sync.dma_start(out=bpar_t[:], in_=t_bpar[:])
            bpg_t = wconst.tile([128, 8], f32, tag="bpg")
            nc.sync.dma_start(out=bpg_t[:], in_=t_bpg[:])

        state = {}   # s -> dict of tiles

        def load_mel(s, chunked=False):
            mc = mpool.tile([MEL_D, W[s] + 2], f16, tag=f"mc{s}", name="mcm")
            if chunked:
                lo = 0
                for (off, n) in mel_chunks[s]:
                    hi = off + n + 2
                    nc.sync.dma_start(out=mc[:, lo:hi], in_=t_mcm[s, :, lo:hi])
                    lo = hi
            else:
                nc.sync.dma_start(out=mc[:], in_=t_mcm[s, :, 0:W[s] + 2])
            vb = vpool.tile([128, W[s] + 2], f16, tag=f"vm{s}", name="vm")
            nc.sync.dma_start(out=vb[:], in_=bcast(t_vm[s, 0:W[s] + 2], 128))
            vb4 = vpool.tile([128, W[s] + 2], f16, tag=f"vm4{s}", name="vm4")
            nc.sync.dma_start(out=vb4[:], in_=bcast(t_vm4[s, 0:W[s] + 2], 128))
            state.setdefault(s, {})
            state[s]["mc"] = mc
            state[s]["vb"] = vb
            state[s]["vb4"] = vb4

        def load_ph(s, eng=None):
            eng = eng or nc.sync
            zt = [zpool.tile([128, SP2], f16, tag=f"z{s}", name="zph")
                  for _ in range(2)]
            for icb in range(2):
                eng.dma_start(out=zt[icb][:], in_=t_zph[s, icb])
            vpb = vppool.tile([128, SP2], f16, tag=f"vp{s}", name="vph")
            eng.dma_start(out=vpb[:], in_=bcast(t_vph[s], 128))
            state.setdefault(s, {})
            state[s]["zt"] = zt
            state[s]["vpb"] = vpb

        def init_conv(s):
            st = state[s]
            mc = st["mc"]
            yt = [ypool.tile([128, YW[s]], f16, tag=f"y{s}", name="y")
                  for _ in range(2)]
            for icb in range(2):
                nc.vector.memset(yt[icb][:, 0:1], 0.0)
                if 1 + W[s] < YW[s]:
                    nc.vector.memset(yt[icb][:, 1 + W[s]:YW[s]], 0.0)
            for (off, n) in mel_chunks[s]:
                for ocb in range(2):
                    pi = ppsum.tile([128, 512], f32, tag="cps", name="cps")
                    for k in range(3):
                        nc.tensor.matmul(
                            pi[:, :n],
                            w0_t[:, k, 128 * ocb:128 * ocb + 128],
                            mc[:, off + k:off + k + n],
                            start=(k == 0), stop=(k == 2 and not has_b0))
                    if has_b0:
                        nc.tensor.matmul(pi[:, :n],
                                         b0_t[0:1, 128 * ocb:128 * ocb + 128],
                                         ones_t[0:1, :n],
                                         start=False, stop=True)
                    nc.vector.tensor_copy(out=yt[ocb][:, off + 1:off + 1 + n],
                                          in_=pi[:, :n])
            st["yt"] = yt

        def glu_block(y_tiles, ym_tag, ym_pool, width, chunks, wt,
                      bar_t, bg_t, blk, vb):
            """One GLU block, channel-major, fp16, in-place on y_tiles."""
            yms = {}
            for icb in range(2):
                for ci, (off, n) in enumerate(chunks):
                    ym = ym_pool.tile([128, width], f16, tag=ym_tag, name=ym_tag)
                    eng = nc.vector if ci < 3 else nc.gpsimd
                    eng.tensor_mul(out=ym[:, :n + 2],
                                   in0=y_tiles[icb][:, off:off + n + 2],
                                   in1=vb[:, off:off + n + 2])
                    yms[(icb, off)] = ym
            for cpair in range(0, len(chunks), 2):
                sub = chunks[cpair:cpair + 2]
                for oco in range(2):
                    pa = {}
                    pg = {}
                    for (off, n) in sub:
                        pa[off] = ppsum.tile([128, 512], f32, tag="cps", name="cps")
                        pg[off] = ppsum.tile([128, 512], f32, tag="cps", name="cps")
                    last_mm = (2, 1)
                    for k in range(3):
                        for icb in range(2):
                            st_ = (k == 0 and icb == 0)
                            sp = ((k, icb) == last_mm and bar_t is None)
                            wa = wt[:, k, icb, 128 * oco:128 * oco + 128]
                            wg = wt[:, k, icb, 256 + 128 * oco:384 + 128 * oco]
                            for (off, n) in sub:
                                nc.tensor.matmul(pa[off][:, :n], wa,
                                                 yms[(icb, off)][:, k:k + n],
                                                 start=st_, stop=sp)
                            for (off, n) in sub:
                                nc.tensor.matmul(pg[off][:, :n], wg,
                                                 yms[(icb, off)][:, k:k + n],
                                                 start=st_,
                                                 stop=((k, icb) == last_mm))
                    if bar_t is not None:
                        for (off, n) in sub:
                            nc.tensor.matmul(pa[off][:, :n],
                                             bar_t[blk:blk + 1, 128 * oco:128 * oco + 128],
                                             ones_t[0:1, :n],
                                             start=False, stop=True)
                    for (off, n) in sub:
                        sig = sgpool.tile([128, 512], f16, tag="sig", name="sig")
                        bias = bg_t[:, 2 * blk + oco:2 * blk + oco + 1] if bg_t is not None else 0.0
                        nc.scalar.activation(out=sig[:, :n], in_=pg[off][:, :n],
                                             func=AF.Sigmoid, bias=bias)
                        nc.vector.tensor_mul(out=sig[:, :n], in0=pa[off][:, :n],
                                             in1=sig[:, :n])
                        nc.vector.tensor_add(out=y_tiles[oco][:, off + 1:off + 1 + n],
                                             in0=sig[:, :n],
                                             in1=yms[(oco, off)][:, 1:1 + n])

        def glu_mel(s, blk):
            st = state[s]
            glu_block(st["yt"], "ym", ympool, 502, mel_chunks[s], wm_t[blk],
                      bmar_t if has_bm else None,
                      bmg_t if has_bm else None, blk,
                      st["vb4"] if blk == 3 else st["vb"])

        def glu_ph(s, blk):
            st = state[s]
            glu_block(st["zt"], "ymp", ymppool, SP2, ph_chunks[s], wp_t[blk],
                      bpar_t if has_bp else None,
                      bpg_t if has_bp else None, blk, st["vpb"])

        def attn_pre(s):
            """After ph GLU of slot s: transpose z, biases, ctx pad row + pad DMA."""
            st = state[s]
            zt = st["zt"]
            n_sb = NSB[s]
            mv_t = spool.tile([128, S_pad // 128], f32, tag=f"mv{s}", name="mv")
            src = t_mv[s]
            nc.sync.dma_start(out=mv_t[:, :n_sb], in_=bass.AP(
                tensor=src.tensor, offset=src.offset,
                ap=[[1, 128], [128, n_sb]]))
            zts = []
            biases = []
            for sb in range(n_sb):
                zp = atpsum.tile([128, 512], f32, tag="atp", name="ztp")
                for dcb in range(2):
                    nc.tensor.matmul(zp[:, 128 * dcb:128 * dcb + 128],
                                     zt[dcb][:, 1 + 128 * sb:129 + 128 * sb],
                                     id_t[:], start=True, stop=True)
                z = ztspool.tile([128, 260], bf16, tag=f"zts{s}", name="zts")
                nc.scalar.copy(out=z[:, 0:256], in_=zp[:, 0:256])
                nc.vector.memset(z[:, 256:260], 1.0)
                sq = sqpool.tile([128, 256], f32, tag="sq", name="sq")
                nc.gpsimd.tensor_mul(out=sq[:], in0=z[:, 0:256], in1=z[:, 0:256])
                ph2 = spool.tile([128, 1], f32, tag="phsq", name="phsq")
                nc.vector.tensor_reduce(out=ph2[:], in_=sq[:], axis=AX.X, op=ALU.add)
                bias_sb = spool.tile([128, 1], f32, tag=f"bias{s}", name="bias")
                nc.vector.tensor_scalar(out=bias_sb[:], in0=ph2[:],
                                        scalar1=-C8, scalar2=mv_t[:, sb:sb + 1],
                                        op0=ALU.mult, op1=ALU.add)
                zts.append(z)
                biases.append(bias_sb)
            st["zts"] = zts
            st["biases"] = biases
            if Tb[s] < T_MEL:
                # ctx pad row (softmax with zero mel row) from the ph side
                # only.  sigmoid(x) == exp(x) to ~e^-19 relative for x <= -19;
                # biases are -C8*|ph|^2 (or -1e9), always << -19, and Sigmoid
                # is the resident table during the GLU stack.
                pp = padpsum.tile([128, 260], f32, tag="padp", name="padp")
                for sb in range(n_sb):
                    eb = spool.tile([128, 1], bf16, tag="eb", name="eb")
                    nc.scalar.activation(out=eb[:], in_=biases[sb][:],
                                         func=AF.Sigmoid)
                    nc.tensor.matmul(pp[0:1, :260], eb[:], zts[sb][:],
                                     start=(sb == 0), stop=(sb == n_sb - 1))
                rc = spool.tile([1, 1], f32, tag="prc", name="prc")
                nc.vector.reciprocal(out=rc[:], in_=pp[0:1, 256:257])
                prow = spool.tile([1, 256], bf16, tag=f"prow{s}", name="prow")
                nc.vector.tensor_scalar(out=prow[:], in0=pp[0:1, 0:256],
                                        scalar1=rc[:], scalar2=C4,
                                        op0=ALU.mult, op1=ALU.mult)
                pr = padpsum.tile([128, 260], f32, tag="padp", name="padp")
                nc.tensor.matmul(pr[:, :256], ones1_t[:], prow[:],
                                 start=True, stop=True)
                padf = padpool.tile([128, 256], f16, tag=f"padf{s}", name="padf")
                nc.vector.tensor_copy(out=padf[:], in_=pr[:, :256])
                for r0 in range(Tb[s], T_MEL, 128):
                    nr = min(128, T_MEL - r0)
                    nc.gpsimd.dma_start(out=t_outc[s, r0:r0 + nr, :],
                                        in_=padf[:nr, :])

        def zero_pad_mel(s):
            if Tb[s] >= T_MEL:
                return
            nz = T_MEL - Tb[s]
            for dcb in range(2):
                nc.gpsimd.dma_start(out=t_outm[s, dcb, :, Tb[s]:T_MEL],
                                    in_=zero_t[:, :nz])

        def dots(s):
            st = state[s]
            yt, zt, biases = st["yt"], st["zt"], st["biases"]
            n_sb = NSB[s]
            ets = []
            for sb in range(n_sb):
                et = epool.tile([128, Tb[s]], bf16, tag=f"et{s}", name="exp")
                for (off, n) in dot_chunks[s]:
                    dp = atpsum.tile([128, 512], f32, tag="atp", name="dps")
                    for dcb in range(2):
                        nc.tensor.matmul(
                            dp[:, :n],
                            zt[dcb][:, 1 + 128 * sb:129 + 128 * sb],
                            yt[dcb][:, 1 + off:1 + off + n],
                            start=(dcb == 0), stop=(dcb == 1))
                    nc.scalar.activation(out=et[:, off:off + n], in_=dp[:, :n],
                                         func=AF.Exp, bias=biases[sb], scale=2.0)
                ets.append(et)
            st["ets"] = ets

        def ctx(s):
            st = state[s]
            ets, zts = st["ets"], st["zts"]
            n_sb = NSB[s]
            for tt in range((Tb[s] + 127) // 128):
                rows = min(128, Tb[s] - 128 * tt)
                cp = atpsum.tile([128, 512], f32, tag="atp", name="cxs")
                for sb in range(n_sb):
                    nc.tensor.matmul(cp[:rows, :260],
                                     ets[sb][:, 128 * tt:128 * tt + rows],
                                     zts[sb][:],
                                     start=(sb == 0), stop=(sb == n_sb - 1))
                rc = spool.tile([128, 1], f32, tag="rc", name="rc")
                nc.vector.reciprocal(out=rc[:rows], in_=cp[:rows, 256:257])
                oc = opool.tile([128, 256], f16, tag="oc", name="oc")
                nc.vector.tensor_scalar(out=oc[:rows, :],
                                        in0=cp[:rows, 0:256],
                                        scalar1=rc[:rows], scalar2=C4,
                                        op0=ALU.mult, op1=ALU.mult)
                eng = nc.scalar if s == SPC - 1 else nc.sync
                eng.dma_start(out=t_outc[s, 128 * tt:128 * tt + rows, :],
                              in_=oc[:rows, :])

        def mel_out(s):
            st = state[s]
            yt = st["yt"]
            eng = nc.scalar if s == SPC - 1 else nc.sync
            for dcb in range(2):
                eng.dma_start(out=t_outm[s, dcb, :, 0:Tb[s]],
                              in_=yt[dcb][:, 1:1 + Tb[s]])

        # ================= schedule =================
        # phase 0: slot-0 mel GLU x ph(0), then a 4-stream round-robin of
        # ph(1..3) + mel(1) so the PE always has an independent stream while
        # each GLU block's sigmoid/mul/add tail drains.  attn_pre(s) runs
        # right after ph(s) finishes so its DVE/Act work spreads out.
        load_mel(0, chunked=True)
        load_weights_head()
        load_ph(0)
        nc.scalar.dma_start(out=wp_t[0][:], in_=t_wp[0])
        nc.scalar.dma_start(out=id_t[:], in_=t_id[:])
        load_mel(1)
        load_weights_rest()
        for s in range(1, SPC):
            load_ph(s)
        init_conv(0)
        for blk in range(4):
            glu_mel(0, blk)
            glu_ph(0, blk)
        attn_pre(0)
        for blk in range(4):
            for s in range(1, SPC):
                glu_ph(s, blk)
                if blk == 3:
                    attn_pre(s)

        # phases 1..3: attn(s-1) overlapped with mel GLU(s)
        for s in range(1, SPC):
            if s + 1 < SPC:
                load_mel(s + 1)
            dots(s - 1)
            init_conv(s)
            mel_out(s - 1)
            glu_mel(s, 0)
            ctx(s - 1)
            if s == 2:
                for s2 in range(SPC):
                    zero_pad_mel(s2)
            for blk in range(1, 4):
                glu_mel(s, blk)
        # tail: attention of the shortest slot
        dots(SPC - 1)
        mel_out(SPC - 1)
        ctx(SPC - 1)

    if not nc.is_finalized():
        nc.finalize()
    return nc


def _get_program(S_pad, L, SL, has_b0, has_bm, has_bp):
    key = (S_pad, L, SL, has_b0, has_bm, has_bp)
    if key not in _prog_cache:
        _prog_cache[key] = _build_program(S_pad, L, SL, has_b0, has_bm, has_bp)
    return _prog_cache[key]


LAST_RESULTS = None


def _install_ntff_hook():
    """Provide antenv.axon_hooks (missing in this image) so trace=True works."""
    import sys
    import types
    import ctypes
    import contextlib
    if "antenv.axon_hooks" in sys.modules:
        return
    try:
        import antenv
    except ImportError:
        return
    mod = types.ModuleType("antenv.axon_hooks")
    state = {}
    mod.set_axon_ntff_profile_hook = lambda h: state.__setitem__("h", h)
    mod.get_axon_ntff_profile_hook = lambda: state.get("h")
    sys.modules["antenv.axon_hooks"] = mod
    antenv.axon_hooks = mod
    so_path = "/opt/axon/libaxon_pjrt.so"
    if not os.path.exists(so_path):
        return
    lib = ctypes.CDLL(so_path)
    if not hasattr(lib, "axon_start_nrt_profile"):
        return
    lib.axon_start_nrt_profile.argtypes = [ctypes.POINTER(ctypes.c_int64),
                                           ctypes.c_size_t]
    lib.axon_start_nrt_profile.restype = ctypes.c_int64
    lib.axon_stop_nrt_profile.argtypes = [ctypes.c_char_p]
    lib.axon_stop_nrt_profile.restype = ctypes.c_int64

    @contextlib.contextmanager
    def _hook(output_dir, device_ids):
        import jax
        jax.devices()
        if device_ids:
            ids = (ctypes.c_int64 * len(device_ids))(*device_ids)
            rc = lib.axon_start_nrt_profile(ids, len(device_ids))
        else:
            rc = lib.axon_start_nrt_profile(None, 0)
        if rc != 0:
            raise RuntimeError(f"axon_start_nrt_profile rc={rc}")
        try:
            yield
        finally:
            n = lib.axon_stop_nrt_profile(str(output_dir).encode())
            print(f"ntff profile: {n} file(s) -> {output_dir}")

    mod.set_axon_ntff_profile_hook(_hook)


def kernel(mels, phonemes, mel_lens, phoneme_lens, embedding,
           mel_conv_w, mel_conv_b, ph_w, ph_b, mel_w, mel_b):
    global LAST_RESULTS
    from concourse.bass_utils import run_bass_kernel_spmd


    mels = np.asarray(mels)
    assert mels.shape == (B, T_MEL, MEL_D), mels.shape
    max_pl = int(np.max(np.asarray(phoneme_lens)))
    S_pad = 512 if max_pl <= 511 else 640

    in_maps, flags, perm, L, SL = _host_prep(
        np.asarray(mels), np.asarray(phonemes), np.asarray(mel_lens),
        np.asarray(phoneme_lens), np.asarray(embedding),
        np.asarray(mel_conv_w), np.asarray(mel_conv_b),
        np.asarray(ph_w), np.asarray(ph_b),
        np.asarray(mel_w), np.asarray(mel_b), S_pad)

    nc = _get_program(S_pad, L, SL, *flags)
    trace = bool(int(os.environ.get("KERNEL_TRACE", "0")))
    if trace:
        _install_ntff_hook()
    res = run_bass_kernel_spmd(nc, in_maps, core_ids=list(range(N_CORES)),
                               trace=trace,
                               tmpdir=os.environ.get("KERNEL_TRACE_DIR"))
    LAST_RESULTS = res
    out = np.empty((B, T_MEL, 512), np.float32)
    for c in range(N_CORES):
        for j in range(SPC):
            b = int(perm[8 * j + c])
            out[b, :, :256] = res.results[c]["outm"][j].reshape(256, T_MEL).T
            out[b, :, 256:] = res.results[c]["outc"][j]
    return out


# revision 42
# speedup vs baseline: 1.0220x; 1.0057x over previous
"""Trainium2 Bass kernel for nn_AligningModel (mel/phoneme GLU encoders + soft attention).

Strategy:
  - Data-parallel over batch: 32 samples -> 8 cores x 4 slots.  The sample ->
    (core, slot) assignment starts length-sorted and is then hill-climbed to
    jointly shrink the per-slot compile-time bounds (max mel len AND max
    phoneme len over the 8 cores sharing a slot).
  - Slot-pipelined schedule: slots processed longest-first; attention of slot s
    overlaps the mel GLU stack of slot s+1, so the exposed attention tail is
    the SHORTEST slot.  All phoneme encoders, softmax pad rows, and the big
    pad-region DMA writes happen early (phase 0 / phase 2).
  - fp16 conv path (weights, activations, residual) / bf16 attention path
    (exp tiles need bf16 range: logits can reach +mel_sq since the softmax
    drops the row-constant mel_sq term).  fp16 outputs, upcast on host.
    PSUM accumulation is fp32.
  - All GLU weights are pre-transposed host-side to [i,k,c,o] so each block is
    ONE contiguous 128x6KB DMA, loaded once for the whole kernel; DMA issue
    order puts the startup-critical tensors first.
  - Channel-major [C,T] layout on-chip so the k=3 convs are plain matmuls; the
    sqrt(0.5)^b block scales are folded into the g-path conv weights, and the
    final C^4 into the output copies.
  - Transposes are plain matmuls against a fp16 identity (output lands f32 in
    PSUM, 1 cycle/row).
  - Z (softmax denominator) via ones-columns appended to the time-major ph
    encoding inside the context matmul; exp bias folds -C8*ph_sq and the
    phoneme mask (-1e9).  The padded-row context value is computed from the
    phoneme side alone using sigmoid(bias) == exp(bias) (bias << -19), keeping
    the Exp activation table out of the GLU phase.
  - Engine discipline: PE never waits on an engine that also issues bulk DMA
    descriptors.  gpsimd(Pool) does overflow mask-muls + pad DMA issue, DVE
    does the PSUM-reading elementwise work + residual adds, Act does
    sigmoid/exp + PSUM->SBUF copies, SP(sync) issues all input loads and
    output row writes.
"""

import os
import numpy as np

B = 32
N_CORES = 8
SPC = 4           # samples (slots) per core
T_MEL = 2000
MEL_D = 80
D = 256
C = float(np.sqrt(0.5))
C4 = 0.25         # C**4 exact
C8 = 0.0625       # C**8 exact

_prog_cache = {}


def _chunks(total, cap):
    """Split `total` into <=cap chunks, each a multiple of 4.
    Prefer equal chunks >=256; else greedy cap + remainder."""
    assert total % 4 == 0 and total > 0
    n = -(-total // cap)
    base = min(cap, ((total + n - 1) // n + 3) // 4 * 4)
    if base < 256:
        base = cap
    out = []
    off = 0
    while off < total:
        w = min(base, total - off)
        out.append((off, w))
        off += w
    return out


def _host_prep(mels, phonemes, mel_lens, phoneme_lens, embedding,
               mel_conv_w, mel_conv_b, ph_w, ph_b, mel_w, mel_b, S_pad):
    """Build the per-core input maps (numpy only). Returns (in_maps, flags,
    perm, L, SL) where perm[8*j + c] = original sample index in core c slot j."""
    f32 = np.float32
    f16 = np.float16
    SP2 = S_pad + 2

    ml = np.asarray(mel_lens)
    pl_ = np.asarray(phoneme_lens)
    order = np.argsort(-ml, kind="stable")
    groups = [list(order[8 * j:8 * j + 8]) for j in range(SPC)]

    def slot_cost(g):
        Lm = int(max(ml[i] for i in g))
        Sp = int(max(pl_[i] for i in g))
        Wj = min(T_MEL, -(-(Lm + 2) // 4) * 4)
        SWj = min(S_pad, -(-(Sp + 2) // 4) * 4)
        Tbj = min(T_MEL, -(-(Lm + 2) // 128) * 128)
        nsb = min(S_pad // 128, -(-(Sp + 2) // 128))
        return 102 * Wj + 96 * SWj + (2 + 4.05 * nsb) * Tbj

    # hill-climb swaps between groups to shrink the summed per-slot bounds
    rng = np.random.default_rng(0)
    cur = [slot_cost(g) for g in groups]
    for _ in range(20000):
        a, b = rng.integers(0, SPC, 2)
        if a == b:
            continue
        ia, ib = int(rng.integers(0, 8)), int(rng.integers(0, 8))
        groups[a][ia], groups[b][ib] = groups[b][ib], groups[a][ia]
        na, nb = slot_cost(groups[a]), slot_cost(groups[b])
        if na + nb < cur[a] + cur[b] - 1e-9:
            cur[a], cur[b] = na, nb
        else:
            groups[a][ia], groups[b][ib] = groups[b][ib], groups[a][ia]
    groups.sort(key=lambda g: -max(ml[i] for i in g))
    perm = np.asarray([groups[j][c] for j in range(SPC) for c in range(8)])
    L = tuple(int(max(ml[i] for i in groups[j])) for j in range(SPC))
    SL = tuple(int(max(pl_[i] for i in groups[j])) for j in range(SPC))

    # w0: [k,i,o] -> [i,k,o] contiguous fp16
    w0 = np.ascontiguousarray(
        np.transpose(mel_conv_w, (2, 1, 0)).astype(f32).transpose(1, 0, 2)
    ).astype(f16)

    def pack_w(w4, fold_c4_last=False):
        out = np.empty((4, 128, 3, 2, 512), f16)
        for b in range(4):
            w = np.transpose(w4[b], (2, 1, 0)).astype(f32)  # [k, i, o]
            w = w.reshape(3, 2, 128, 512)
            w[:, :, :, 256:] *= f32(C ** b)
            if fold_c4_last and b == 3:
                # block-3 inputs arrive pre-scaled by C4 (vm4 mask); undo on
                # the g path so sigmoid sees the true gate values.
                w[:, :, :, 256:] *= f32(1.0 / C4)
            out[b] = w.transpose(2, 0, 1, 3).astype(f16)    # [i, k, c, o]
        return np.ascontiguousarray(out)
    wm = pack_w(mel_w, fold_c4_last=True)
    wp = pack_w(ph_w)
    idc = np.eye(128, dtype=f16)

    has_b0 = bool(np.any(mel_conv_b))
    has_bm = bool(np.any(mel_b))
    has_bp = bool(np.any(ph_b))
    shared = {"w0": w0, "wm": wm, "wp": wp, "idc": idc}
    if has_b0:
        shared["b0r"] = mel_conv_b.astype(f32).reshape(1, 256)
    if has_bm:
        bmar_h = mel_b[:, :256].astype(f32).copy()
        bmar_h[3] *= f32(C4)
        shared["bmar"] = bmar_h
        shared["bmg"] = np.ascontiguousarray(
            mel_b[:, 256:].astype(f32).reshape(4, 2, 128).transpose(2, 0, 1).reshape(128, 8))
    if has_bp:
        shared["bpar"] = ph_b[:, :256].astype(f32)
        shared["bpg"] = np.ascontiguousarray(
            ph_b[:, 256:].astype(f32).reshape(4, 2, 128).transpose(2, 0, 1).reshape(128, 8))

    ar = np.arange(T_MEL)
    ars = np.arange(S_pad)
    in_maps = []
    for c in range(N_CORES):
        idx = [int(perm[8 * j + c]) for j in range(SPC)]
        m = dict(shared)
        mcm = np.zeros((SPC, MEL_D, T_MEL + 2), f16)
        vm = np.zeros((SPC, T_MEL + 2), f16)
        vm4 = np.zeros((SPC, T_MEL + 2), f16)
        zph = np.zeros((SPC, 2, 128, SP2), f16)
        vph = np.zeros((SPC, SP2), f16)
        mv = np.full((SPC, S_pad), -1e9, f32)
        for j, b in enumerate(idx):
            mcm[j, :, 1:T_MEL + 1] = np.asarray(mels[b], f32).T.astype(f16)
            vm[j, 1:T_MEL + 1] = (ar < int(mel_lens[b])).astype(f16)
            vm4[j] = vm[j] * f16(C4)
            pl = int(phoneme_lens[b])
            ph_pad = np.concatenate([[0], np.asarray(phonemes[b], np.int64)])[:S_pad]
            e = embedding[ph_pad].astype(f32)
            valid = (ars[:len(e)] <= pl)
            e[~valid] = 0.0
            zph[j, :, :, 1:1 + len(e)] = e.T.reshape(2, 128, len(e)).astype(f16)
            vph[j, 1:1 + len(e)] = valid.astype(f16)
            mv[j, :len(e)][valid] = 0.0
        m["mels_cm"] = mcm
        m["valid_mel"] = vm
        m["valid_mel4"] = vm4
        m["zph0"] = zph
        m["valid_ph"] = vph
        m["mvec"] = mv
        in_maps.append(m)
    return in_maps, (has_b0, has_bm, has_bp), perm, L, SL


def _build_program(S_pad, L, SL, has_b0, has_bm, has_bp):
    from contextlib import ExitStack
    import concourse.bass as bass
    import concourse.bacc as bacc
    import concourse.tile as tile
    from concourse import mybir

    f32 = mybir.dt.float32
    f16 = mybir.dt.float16
    bf16 = mybir.dt.bfloat16
    AF = mybir.ActivationFunctionType
    ALU = mybir.AluOpType
    AX = mybir.AxisListType
    SP2 = S_pad + 2

    # per-slot compile-time bounds (slots sorted longest-first)
    W = [min(T_MEL, -(-(L[j] + 2) // 4) * 4) for j in range(SPC)]       # mel conv cols
    Tb = [min(T_MEL, -(-(L[j] + 2) // 128) * 128) for j in range(SPC)]  # attn rows
    SW = [min(S_pad, -(-(SL[j] + 2) // 4) * 4) for j in range(SPC)]     # ph conv cols
    NSB = [min(S_pad // 128, -(-(SL[j] + 2) // 128)) for j in range(SPC)]
    mel_chunks = [_chunks(W[j], 500 if W[j] > 1000 else
                          (-(--(-W[j] // 4) // 4) * 4 if W[j] > 800 else
                           -(--(-W[j] // 2) // 4) * 4))
                  for j in range(SPC)]
    ph_chunks = [_chunks(SW[j], 512) for j in range(SPC)]
    dot_chunks = [_chunks(Tb[j], 512) for j in range(SPC)]
    YW = [max(W[j], Tb[j]) + 2 for j in range(SPC)]

    nc = bacc.Bacc()
    t_mcm = nc.dram_tensor("mels_cm", [SPC, MEL_D, T_MEL + 2], f16, kind="ExternalInput")
    t_vm = nc.dram_tensor("valid_mel", [SPC, T_MEL + 2], f16, kind="ExternalInput")
    t_vm4 = nc.dram_tensor("valid_mel4", [SPC, T_MEL + 2], f16, kind="ExternalInput")
    t_zph = nc.dram_tensor("zph0", [SPC, 2, 128, SP2], f16, kind="ExternalInput")
    t_vph = nc.dram_tensor("valid_ph", [SPC, SP2], f16, kind="ExternalInput")
    t_mv = nc.dram_tensor("mvec", [SPC, S_pad], f32, kind="ExternalInput")
    t_w0 = nc.dram_tensor("w0", [MEL_D, 3, 256], f16, kind="ExternalInput")
    t_wm = nc.dram_tensor("wm", [4, 128, 3, 2, 512], f16, kind="ExternalInput")
    t_wp = nc.dram_tensor("wp", [4, 128, 3, 2, 512], f16, kind="ExternalInput")
    t_id = nc.dram_tensor("idc", [128, 128], f16, kind="ExternalInput")
    t_b0 = nc.dram_tensor("b0r", [1, 256], f32, kind="ExternalInput") if has_b0 else None
    t_bmar = nc.dram_tensor("bmar", [4, 256], f32, kind="ExternalInput") if has_bm else None
    t_bmg = nc.dram_tensor("bmg", [128, 8], f32, kind="ExternalInput") if has_bm else None
    t_bpar = nc.dram_tensor("bpar", [4, 256], f32, kind="ExternalInput") if has_bp else None
    t_bpg = nc.dram_tensor("bpg", [128, 8], f32, kind="ExternalInput") if has_bp else None
    t_outm = nc.dram_tensor("outm", [SPC, 2, 128, T_MEL], f16, kind="ExternalOutput")
    t_outc = nc.dram_tensor("outc", [SPC, T_MEL, 256], f16, kind="ExternalOutput")

    def bcast(ap, parts):
        return bass.AP(tensor=ap.tensor, offset=ap.offset, ap=[[0, parts]] + list(ap.ap))

    with tile.TileContext(nc) as tc, ExitStack() as ctx:
        wconst = ctx.enter_context(tc.tile_pool(name="wconst", bufs=1))
        ypool = ctx.enter_context(tc.tile_pool(name="y", bufs=2))
        zpool = ctx.enter_context(tc.tile_pool(name="zph", bufs=2))
        vpool = ctx.enter_context(tc.tile_pool(name="vm", bufs=1))
        vppool = ctx.enter_context(tc.tile_pool(name="vph", bufs=1))
        mpool = ctx.enter_context(tc.tile_pool(name="mcm", bufs=1))
        ympool = ctx.enter_context(tc.tile_pool(name="ym", bufs=12))
        ymppool = ctx.enter_context(tc.tile_pool(name="ymp", bufs=6))
        sgpool = ctx.enter_context(tc.tile_pool(name="sig", bufs=8))
        epool = ctx.enter_context(tc.tile_pool(name="exp", bufs=S_pad // 128))
        ztspool = ctx.enter_context(tc.tile_pool(name="zts", bufs=S_pad // 128))
        sqpool = ctx.enter_context(tc.tile_pool(name="sq", bufs=2))
        spool = ctx.enter_context(tc.tile_pool(name="small", bufs=S_pad // 128))
        opool = ctx.enter_context(tc.tile_pool(name="octx", bufs=3))
        padpool = ctx.enter_context(tc.tile_pool(name="padf", bufs=1))
        ppsum = ctx.enter_context(tc.tile_pool(name="pconv", bufs=5, space="PSUM"))
        atpsum = ctx.enter_context(tc.tile_pool(name="pattn", bufs=2, space="PSUM"))
        padpsum = ctx.enter_context(tc.tile_pool(name="ppad", bufs=1, space="PSUM"))

        # ---- constants / weights (weights loaded once; DMA order puts the
        # critical path first: mels(0) -> w0 -> wm/wp block by block) ----
        wm_t = [wconst.tile([128, 3, 2, 512], f16, tag=f"wm{b}", name="wm")
                for b in range(4)]
        wp_t = [wconst.tile([128, 3, 2, 512], f16, tag=f"wp{b}", name="wp")
                for b in range(4)]
        id_t = wconst.tile([128, 128], f16, tag="id")
        w0_t = wconst.tile([MEL_D, 3, 256], f16, tag="w0")
        zero_t = wconst.tile([128, 1536], f16, tag="zero")
        nc.vector.memset(zero_t[:], 0.0)
        ones1_t = wconst.tile([1, 128], bf16, tag="ones1")
        nc.vector.memset(ones1_t[:], 1.0)

        def load_weights_head():
            # separate hw queue from the mel loads so both ramp concurrently
            nc.scalar.dma_start(out=w0_t[:], in_=t_w0[:])
            nc.scalar.dma_start(out=wm_t[0][:], in_=t_wm[0])

        def load_weights_rest():
            for b in range(1, 4):
                nc.sync.dma_start(out=wm_t[b][:], in_=t_wm[b])
                nc.sync.dma_start(out=wp_t[b][:], in_=t_wp[b])

        need_ones = has_b0 or has_bm or has_bp
        if need_ones:
            ones_t = wconst.tile([1, 512], f32, tag="ones")
            nc.vector.memset(ones_t[:], 1.0)
        if has_b0:
            b0_t = wconst.tile([1, 256], f32, tag="b0")
            nc.sync.dma_start(out=b0_t[:], in_=t_b0[:])
        if has_bm:
            bmar_t = wconst.tile([4, 256], f32, tag="bmar")
            nc.sync.dma_start(out=bmar_t[:], in_=t_bmar[:])
            bmg_t = wconst.tile([128, 8], f32, tag="bmg")
            nc.sync.dma_start(out=bmg_t[:], in_=t_bmg[:])
        if has_bp:
            bpar_t = wconst.tile([4, 256], f32, tag="bpar")
            nc.sync.dma_start(out=bpar_t[:], in_=t_bpar[:])
            bpg_t = wconst.tile([128, 8], f32, tag="bpg")
            nc.sync.dma_start(out=bpg_t[:], in_=t_bpg[:])

        state = {}   # s -> dict of tiles

        def load_mel(s, chunked=False):
            mc = mpool.tile([MEL_D, W[s] + 2], f16, tag=f"mc{s}", name="mcm")
            if chunked:
                lo = 0
                for (off, n) in mel_chunks[s]:
                    hi = off + n + 2
                    nc.sync.dma_start(out=mc[:, lo:hi], in_=t_mcm[s, :, lo:hi])
                    lo = hi
            else:
                nc.sync.dma_start(out=mc[:], in_=t_mcm[s, :, 0:W[s] + 2])
            vb = vpool.tile([128, W[s] + 2], f16, tag=f"vm{s}", name="vm")
            nc.sync.dma_start(out=vb[:], in_=bcast(t_vm[s, 0:W[s] + 2], 128))
            vb4 = vpool.tile([128, W[s] + 2], f16, tag=f"vm4{s}", name="vm4")
            nc.sync.dma_start(out=vb4[:], in_=bcast(t_vm4[s, 0:W[s] + 2], 128))
            state.setdefault(s, {})
            state[s]["mc"] = mc
            state[s]["vb"] = vb
            state[s]["vb4"] = vb4

        def load_ph(s, eng=None):
            eng = eng or nc.sync
            zt = [zpool.tile([128, SP2], f16, tag=f"z{s}", name="zph")
                  for _ in range(2)]
            for icb in range(2):
                eng.dma_start(out=zt[icb][:], in_=t_zph[s, icb])
            vpb = vppool.tile([128, SP2], f16, tag=f"vp{s}", name="vph")
            eng.dma_start(out=vpb[:], in_=bcast(t_vph[s], 128))
            state.setdefault(s, {})
            state[s]["zt"] = zt
            state[s]["vpb"] = vpb

        def init_conv(s):
            st = state[s]
            mc = st["mc"]
            yt = [ypool.tile([128, YW[s]], f16, tag=f"y{s}", name="y")
                  for _ in range(2)]
            for icb in range(2):
                nc.vector.memset(yt[icb][:, 0:1], 0.0)
                if 1 + W[s] < YW[s]:
                    nc.vector.memset(yt[icb][:, 1 + W[s]:YW[s]], 0.0)
            for (off, n) in mel_chunks[s]:
                for ocb in range(2):
                    pi = ppsum.tile([128, 512], f32, tag="cps", name="cps")
                    for k in range(3):
                        nc.tensor.matmul(
                            pi[:, :n],
                            w0_t[:, k, 128 * ocb:128 * ocb + 128],
                            mc[:, off + k:off + k + n],
                            start=(k == 0), stop=(k == 2 and not has_b0))
                    if has_b0:
                        nc.tensor.matmul(pi[:, :n],
                                         b0_t[0:1, 128 * ocb:128 * ocb + 128],
                                         ones_t[0:1, :n],
                                         start=False, stop=True)
                    nc.vector.tensor_copy(out=yt[ocb][:, off + 1:off + 1 + n],
                                          in_=pi[:, :n])
            st["yt"] = yt

        def glu_block(y_tiles, ym_tag, ym_pool, width, chunks, wt,
                      bar_t, bg_t, blk, vb):
            """One GLU block, channel-major, fp16, in-place on y_tiles."""
            yms = {}
            for icb in range(2):
                for ci, (off, n) in enumerate(chunks):
                    ym = ym_pool.tile([128, width], f16, tag=ym_tag, name=ym_tag)
                    eng = nc.vector if ci < 3 else nc.gpsimd
                    eng.tensor_mul(out=ym[:, :n + 2],
                                   in0=y_tiles[icb][:, off:off + n + 2],
                                   in1=vb[:, off:off + n + 2])
                    yms[(icb, off)] = ym
            for cpair in range(0, len(chunks), 2):
                sub = chunks[cpair:cpair + 2]
                for oco in range(2):
                    pa = {}
                    pg = {}
                    for (off, n) in sub:
                        pa[off] = ppsum.tile([128, 512], f32, tag="cps", name="cps")
                        pg[off] = ppsum.tile([128, 512], f32, tag="cps", name="cps")
                    last_mm = (2, 1)
                    for k in range(3):
                        for icb in range(2):
                            st_ = (k == 0 and icb == 0)
                            sp = ((k, icb) == last_mm and bar_t is None)
                            wa = wt[:, k, icb, 128 * oco:128 * oco + 128]
                            wg = wt[:, k, icb, 256 + 128 * oco:384 + 128 * oco]
                            for (off, n) in sub:
                                nc.tensor.matmul(pa[off][:, :n], wa,
                                                 yms[(icb, off)][:, k:k + n],
                                                 start=st_, stop=sp)
                            for (off, n) in sub:
                                nc.tensor.matmul(pg[off][:, :n], wg,
                                                 yms[(icb, off)][:, k:k + n],
                                                 start=st_,
                                                 stop=((k, icb) == last_mm))
                    if bar_t is not None:
                        for (off, n) in sub:
                            nc.tensor.matmul(pa[off][:, :n],
                                             bar_t[blk:blk + 1, 128 * oco:128 * oco + 128],
                                             ones_t[0:1, :n],
                                             start=False, stop=True)
                    for (off, n) in sub:
                        sig = sgpool.tile([128, 512], f16, tag="sig", name="sig")
                        bias = bg_t[:, 2 * blk + oco:2 * blk + oco + 1] if bg_t is not None else 0.0
                        nc.scalar.activation(out=sig[:, :n], in_=pg[off][:, :n],
                                             func=AF.Sigmoid, bias=bias)
                        nc.vector.tensor_mul(out=sig[:, :n], in0=pa[off][:, :n],
                                             in1=sig[:, :n])
                        nc.vector.tensor_add(out=y_tiles[oco][:, off + 1:off + 1 + n],
                                             in0=sig[:, :n],
                                             in1=yms[(oco, off)][:, 1:1 + n])

        def glu_mel(s, blk):
            st = state[s]
            glu_block(st["yt"], "ym", ympool, 502, mel_chunks[s], wm_t[blk],
                      bmar_t if has_bm else None,
                      bmg_t if has_bm else None, blk,
                      st["vb4"] if blk == 3 else st["vb"])

        def glu_ph(s, blk):
            st = state[s]
            glu_block(st["zt"], "ymp", ymppool, SP2, ph_chunks[s], wp_t[blk],
                      bpar_t if has_bp else None,
                      bpg_t if has_bp else None, blk, st["vpb"])

        def attn_pre(s):
            """After ph GLU of slot s: transpose z, biases, ctx pad row + pad DMA."""
            st = state[s]
            zt = st["zt"]
            n_sb = NSB[s]
            mv_t = spool.tile([128, S_pad // 128], f32, tag=f"mv{s}", name="mv")
            src = t_mv[s]
            nc.sync.dma_start(out=mv_t[:, :n_sb], in_=bass.AP(
                tensor=src.tensor, offset=src.offset,
                ap=[[1, 128], [128, n_sb]]))
            zts = []
            biases = []
            for sb in range(n_sb):
                zp = atpsum.tile([128, 512], f32, tag="atp", name="ztp")
                for dcb in range(2):
                    nc.tensor.matmul(zp[:, 128 * dcb:128 * dcb + 128],
                                     zt[dcb][:, 1 + 128 * sb:129 + 128 * sb],
                                     id_t[:], start=True, stop=True)
                z = ztspool.tile([128, 260], bf16, tag=f"zts{s}", name="zts")
                nc.scalar.copy(out=z[:, 0:256], in_=zp[:, 0:256])
                nc.vector.memset(z[:, 256:260], 1.0)
                sq = sqpool.tile([128, 256], f32, tag="sq", name="sq")
                nc.gpsimd.tensor_mul(out=sq[:], in0=z[:, 0:256], in1=z[:, 0:256])
                ph2 = spool.tile([128, 1], f32, tag="phsq", name="phsq")
                nc.vector.tensor_reduce(out=ph2[:], in_=sq[:], axis=AX.X, op=ALU.add)
                bias_sb = spool.tile([128, 1], f32, tag=f"bias{s}", name="bias")
                nc.vector.tensor_scalar(out=bias_sb[:], in0=ph2[:],
                                        scalar1=-C8, scalar2=mv_t[:, sb:sb + 1],
                                        op0=ALU.mult, op1=ALU.add)
                zts.append(z)
                biases.append(bias_sb)
            st["zts"] = zts
            st["biases"] = biases
            if Tb[s] < T_MEL:
                # ctx pad row (softmax with zero mel row) from the ph side
                # only.  sigmoid(x) == exp(x) to ~e^-19 relative for x <= -19;
                # biases are -C8*|ph|^2 (or -1e9), always << -19, and Sigmoid
                # is the resident table during the GLU stack.
                pp = padpsum.tile([128, 260], f32, tag="padp", name="padp")
                for sb in range(n_sb):
                    eb = spool.tile([128, 1], bf16, tag="eb", name="eb")
                    nc.scalar.activation(out=eb[:], in_=biases[sb][:],
                                         func=AF.Sigmoid)
                    nc.tensor.matmul(pp[0:1, :260], eb[:], zts[sb][:],
                                     start=(sb == 0), stop=(sb == n_sb - 1))
                rc = spool.tile([1, 1], f32, tag="prc", name="prc")
                nc.vector.reciprocal(out=rc[:], in_=pp[0:1, 256:257])
                prow = spool.tile([1, 256], bf16, tag=f"prow{s}", name="prow")
                nc.vector.tensor_scalar(out=prow[:], in0=pp[0:1, 0:256],
                                        scalar1=rc[:], scalar2=C4,
                                        op0=ALU.mult, op1=ALU.mult)
                pr = padpsum.tile([128, 260], f32, tag="padp", name="padp")
                nc.tensor.matmul(pr[:, :256], ones1_t[:], prow[:],
                                 start=True, stop=True)
                padf = padpool.tile([128, 256], f16, tag=f"padf{s}", name="padf")
                nc.vector.tensor_copy(out=padf[:], in_=pr[:, :256])
                for r0 in range(Tb[s], T_MEL, 128):
                    nr = min(128, T_MEL - r0)
                    nc.gpsimd.dma_start(out=t_outc[s, r0:r0 + nr, :],
                                        in_=padf[:nr, :])

        def zero_pad_mel(s):
            for dcb in range(2):
                for c0 in range(Tb[s], T_MEL, 1536):
                    nz = min(1536, T_MEL - c0)
                    nc.gpsimd.dma_start(out=t_outm[s, dcb, :, c0:c0 + nz],
                                        in_=zero_t[:, :nz])

        def dots(s):
            st = state[s]
            yt, zt, biases = st["yt"], st["zt"], st["biases"]
            n_sb = NSB[s]
            ets = []
            for sb in range(n_sb):
                et = epool.tile([128, Tb[s]], bf16, tag=f"et{s}", name="exp")
                for (off, n) in dot_chunks[s]:
                    dp = atpsum.tile([128, 512], f32, tag="atp", name="dps")
                    for dcb in range(2):
                        nc.tensor.matmul(
                            dp[:, :n],
                            zt[dcb][:, 1 + 128 * sb:129 + 128 * sb],
                            yt[dcb][:, 1 + off:1 + off + n],
                            start=(dcb == 0), stop=(dcb == 1))
                    nc.scalar.activation(out=et[:, off:off + n], in_=dp[:, :n],
                                         func=AF.Exp, bias=biases[sb], scale=2.0)
                ets.append(et)
            st["ets"] = ets

        def ctx(s):
            st = state[s]
            ets, zts = st["ets"], st["zts"]
            n_sb = NSB[s]
            for tt in range((Tb[s] + 127) // 128):
                rows = min(128, Tb[s] - 128 * tt)
                cp = atpsum.tile([128, 512], f32, tag="atp", name="cxs")
                for sb in range(n_sb):
                    nc.tensor.matmul(cp[:rows, :260],
                                     ets[sb][:, 128 * tt:128 * tt + rows],
                                     zts[sb][:],
                                     start=(sb == 0), stop=(sb == n_sb - 1))
                rc = spool.tile([128, 1], f32, tag="rc", name="rc")
                nc.vector.reciprocal(out=rc[:rows], in_=cp[:rows, 256:257])
                oc = opool.tile([128, 256], f16, tag="oc", name="oc")
                nc.vector.tensor_scalar(out=oc[:rows, :],
                                        in0=cp[:rows, 0:256],
                                        scalar1=rc[:rows], scalar2=C4,
                                        op0=ALU.mult, op1=ALU.mult)
                eng = nc.scalar if s == SPC - 1 else nc.sync
                eng.dma_start(out=t_outc[s, 128 * tt:128 * tt + rows, :],
                              in_=oc[:rows, :])

        def mel_out(s):
            st = state[s]
            yt = st["yt"]
            eng = nc.scalar if s == SPC - 1 else nc.sync
            for dcb in range(2):
                eng.dma_start(out=t_outm[s, dcb, :, 0:Tb[s]],
                              in_=yt[dcb][:, 1:1 + Tb[s]])

        # ================= schedule =================
        # phase 0: slot-0 mel GLU x ph(0), then a 4-stream round-robin of
        # ph(1..3) + mel(1) so the PE always has an independent stream while
        # each GLU block's sigmoid/mul/add tail drains.  attn_pre(s) runs
        # right after ph(s) finishes so its DVE/Act work spreads out.
        load_mel(0, chunked=True)
        load_weights_head()
        load_ph(0)
        nc.scalar.dma_start(out=wp_t[0][:], in_=t_wp[0])
        nc.scalar.dma_start(out=id_t[:], in_=t_id[:])
        load_mel(1)
        load_weights_rest()
        for s in range(1, SPC):
            load_ph(s)
        init_conv(0)
        for blk in range(4):
            glu_mel(0, blk)
            glu_ph(0, blk)
        attn_pre(0)
        for blk in range(4):
            for s in range(1, SPC):
                glu_ph(s, blk)
                if blk == 3:
                    attn_pre(s)

        # phases 1..3: attn(s-1) overlapped with mel GLU(s)
        for s in range(1, SPC):
            if s + 1 < SPC:
                load_mel(s + 1)
            dots(s - 1)
            init_conv(s)
            mel_out(s - 1)
            glu_mel(s, 0)
            ctx(s - 1)
            if s == 2:
                for s2 in range(SPC):
                    zero_pad_mel(s2)
            for blk in range(1, 4):
                glu_mel(s, blk)
        # tail: attention of the shortest slot
        dots(SPC - 1)
        mel_out(SPC - 1)
        ctx(SPC - 1)

    if not nc.is_finalized():
        nc.finalize()
    return nc


def _get_program(S_pad, L, SL, has_b0, has_bm, has_bp):
    key = (S_pad, L, SL, has_b0, has_bm, has_bp)
    if key not in _prog_cache:
        _prog_cache[key] = _build_program(S_pad, L, SL, has_b0, has_bm, has_bp)
    return _prog_cache[key]


LAST_RESULTS = None


def _install_ntff_hook():
    """Provide antenv.axon_hooks (missing in this image) so trace=True works."""
    import sys
    import types
    import ctypes
    import contextlib
    if "antenv.axon_hooks" in sys.modules:
        return
    try:
        import antenv
    except ImportError:
        return
    mod = types.ModuleType("antenv.axon_hooks")
    state = {}
    mod.set_axon_ntff_profile_hook = lambda h: state.__setitem__("h", h)
    mod.get_axon_ntff_profile_hook = lambda: state.get("h")
    sys.modules["antenv.axon_hooks"] = mod
    antenv.axon_hooks = mod
    so_path = "/opt/axon/libaxon_pjrt.so"
    if not os.path.exists(so_path):
        return
    lib = ctypes.CDLL(so_path)
    if not hasattr(lib, "axon_start_nrt_profile"):
        return
    lib.axon_start_nrt_profile.argtypes = [ctypes.POINTER(ctypes.c_int64),
                                           ctypes.c_size_t]
    lib.axon_start_nrt_profile.restype = ctypes.c_int64
    lib.axon_stop_nrt_profile.argtypes = [ctypes.c_char_p]
    lib.axon_stop_nrt_profile.restype = ctypes.c_int64

    @contextlib.contextmanager
    def _hook(output_dir, device_ids):
        import jax
        jax.devices()
        if device_ids:
            ids = (ctypes.c_int64 * len(device_ids))(*device_ids)
            rc = lib.axon_start_nrt_profile(ids, len(device_ids))
        else:
            rc = lib.axon_start_nrt_profile(None, 0)
        if rc != 0:
            raise RuntimeError(f"axon_start_nrt_profile rc={rc}")
        try:
            yield
        finally:
            n = lib.axon_stop_nrt_profile(str(output_dir).encode())
            print(f"ntff profile: {n} file(s) -> {output_dir}")

    mod.set_axon_ntff_profile_hook(_hook)


def kernel(mels, phonemes, mel_lens, phoneme_lens, embedding,
           mel_conv_w, mel_conv_b, ph_w, ph_b, mel_w, mel_b):
    global LAST_RESULTS
    from concourse.bass_utils import run_bass_kernel_spmd


    mels = np.asarray(mels)
    assert mels.shape == (B, T_MEL, MEL_D), mels.shape
    max_pl = int(np.max(np.asarray(phoneme_lens)))
    S_pad = 512 if max_pl <= 511 else 640

    in_maps, flags, perm, L, SL = _host_prep(
        np.asarray(mels), np.asarray(phonemes), np.asarray(mel_lens),
        np.asarray(phoneme_lens), np.asarray(embedding),
        np.asarray(mel_conv_w), np.asarray(mel_conv_b),
        np.asarray(ph_w), np.asarray(ph_b),
        np.asarray(mel_w), np.asarray(mel_b), S_pad)

    nc = _get_program(S_pad, L, SL, *flags)
    trace = bool(int(os.environ.get("KERNEL_TRACE", "0")))
    if trace:
        _install_ntff_hook()
    res = run_bass_kernel_spmd(nc, in_maps, core_ids=list(range(N_CORES)),
                               trace=trace,
                               tmpdir=os.environ.get("KERNEL_TRACE_DIR"))
    LAST_RESULTS = res
    out = np.empty((B, T_MEL, 512), np.float32)
    for c in range(N_CORES):
        for j in range(SPC):
            b = int(perm[8 * j + c])
            out[b, :, :256] = res.results[c]["outm"][j].reshape(256, T_MEL).T
            out[b, :, 256:] = res.results[c]["outc"][j]
    return out
